# revision 8
# baseline (speedup 1.0000x reference)
"""Trainium2 Bass kernel for causal MultiHeadAttention + residual + LayerNorm.

Problem shapes (hardcoded):
  B=4, S=2048, D_MODEL=1024, H=8 heads, d_k=128.
  out = LayerNorm(queries + MHA(LN-free)(queries, keys, values))

Sharding (8 cores):
  Launch 1 (attention): core c <-> (batch b = c//2, head group g = c%2 -> heads
  4g..4g+3).  Q/K/V weights column-sharded by head group; X^T passed
  pre-transposed in bf16.  Each core computes its 4 heads' attention output
  O^T [4,128,2048] f32.
  Launch 2 (layernorm): row-sharded, 1024 rows of the flattened [8192,1024]
  residual per core.
"""

import sys

import numpy as np

for _p in ("/opt/trn_rl_repo", "/opt/pypackages"):
    if _p not in sys.path:
        sys.path.append(_p)

import ml_dtypes  # noqa: E402

import concourse.bass as bass  # noqa: E402
import concourse.mybir as mybir  # noqa: E402
import concourse.tile as tile_mod  # noqa: E402
from concourse.tile import TileContext  # noqa: E402
from concourse.bass_utils import run_bass_kernel_spmd  # noqa: E402
from concourse.masks import make_lower_triangular  # noqa: E402

B = 4
S = 2048
D = 1024
H = 8
DK = 128
HG = 4  # heads per core
NCORES = 8
SCALE = 1.0 / np.sqrt(np.float32(DK))
NEG_INF = -1e9
EPS = 1e-6

BF16 = mybir.dt.bfloat16
F32 = mybir.dt.float32
NPBF16 = ml_dtypes.bfloat16

_PATCHED = False


def _bcast_rows(ap):
    """Broadcast a 1-D dram AP across 128 partitions (step-0 partition dim)."""
    return bass.AP(tensor=ap.tensor, offset=ap.offset, ap=[[0, 128]] + list(ap.ap))


def _patch_tile_drain():
    # retained for API compatibility; wait splitting now happens in
    # _split_excess_waits after scheduling.
    return


def _split_excess_waits(nc):
    """Workaround for this walrus build: engine (TPB) instructions accept at
    most one sync-wait command (EventSemaphore: two), but Tile attaches one
    wait per dependency.  Move excess waits onto same-engine NOPs inserted
    immediately before the over-limit instruction — the engine executes
    in-order, so stalling at the NOP(s) first is semantically identical.
    DMA/collective instructions are exempt (queue descriptors support
    multiple waits)."""
    n_new = 0
    for f in nc.m.functions:
        for bb in f.blocks:
            il = bb.instructions
            out = []
            changed = False
            for ins in il:
                si = ins.sync_info
                tname = type(ins).__name__
                if si is not None:
                    cap = 2 if tname == "InstEventSemaphore" else 1
                    waits = list(si.on_wait)
                    if len(waits) > cap:
                        for w in waits[cap:]:
                            nop = mybir.InstNoOp(
                                name=f"I-wsplit-{n_new}",
                                sync_info=mybir.SyncInfo(
                                    on_wait=[w], on_update=[]
                                ),
                                bass_nofuse=True,
                                engine=ins.engine,
                            )
                            n_new += 1
                            out.append(nop)
                        si.on_wait = waits[:cap]
                        changed = True
                out.append(ins)
            if changed:
                il[:] = out
    return n_new


def _build_attention():
    """Per-core attention program: 4 heads of one batch.

    Inputs (bf16 unless noted):
      xq_t, xk_t, xv_t : [D, S]    transposed activations for this batch
      wq, wk, wv       : [D, 4*DK] weight column-slices for this head group
      bq, bk, bv       : [4*DK]    f32 bias slices
    Output:
      o_t : [HG, DK, S] f32 -- per-head normalized attention output, transposed.
    """
    nc = bass.Bass()

    xq_t = nc.dram_tensor("xq_t", [D, S], BF16, kind="ExternalInput")
    xk_t = nc.dram_tensor("xk_t", [D, S], BF16, kind="ExternalInput")
    xv_t = nc.dram_tensor("xv_t", [D, S], BF16, kind="ExternalInput")
    wq = nc.dram_tensor("wq", [D, HG * DK], BF16, kind="ExternalInput")
    wk = nc.dram_tensor("wk", [D, HG * DK], BF16, kind="ExternalInput")
    wv = nc.dram_tensor("wv", [D, HG * DK], BF16, kind="ExternalInput")
    bq = nc.dram_tensor("bq", [HG * DK], F32, kind="ExternalInput")
    bk = nc.dram_tensor("bk", [HG * DK], F32, kind="ExternalInput")
    bv = nc.dram_tensor("bv", [HG * DK], F32, kind="ExternalInput")
    o_t = nc.dram_tensor("o_t", [HG, DK, S], F32, kind="ExternalOutput")

    KC = D // 128          # 8 contraction chunks
    NS = S // 512          # 4 s-chunks of 512
    NJ = S // 128          # 16 key chunks
    HW = HG * DK           # 512

    with TileContext(nc) as tc:
        from contextlib import ExitStack

        with ExitStack() as ctx:
            consts = ctx.enter_context(tc.tile_pool(name="consts", bufs=1))
            proj_out = ctx.enter_context(tc.tile_pool(name="proj_out", bufs=1))

            # --- constants ---
            tril = consts.tile([128, 128], F32)  # additive: -1e9 where k > q
            make_lower_triangular(nc, tril, val=NEG_INF, diag=False)
            ones_l = consts.tile([128, 128], BF16)
            nc.vector.memset(ones_l, 1.0)
            # per-partition bias views: [(h p) -> p h]
            bq_sb = consts.tile([128, HG], F32)
            bk_sb = consts.tile([128, HG], F32)
            nc.gpsimd.dma_start(out=bq_sb, in_=bq.rearrange("(h p) -> p h", p=128))
            nc.gpsimd.dma_start(out=bk_sb, in_=bk.rearrange("(h p) -> p h", p=128))
            # bv broadcast across partitions: [128, 512]
            bv_sb = consts.tile([128, HW], F32)
            nc.gpsimd.dma_start(
                out=bv_sb,
                in_=_bcast_rows(bv[:]),
            )

            # --- projection outputs (live through attention) ---
            qt_sb = [proj_out.tile([128, S], BF16, tag=f"qt{h}", name=f"qt{h}") for h in range(HG)]
            kt_sb = [proj_out.tile([128, S], BF16, tag=f"kt{h}", name=f"kt{h}") for h in range(HG)]
            v_sb = proj_out.tile([128, NJ, HW], BF16, tag="v", name="v")

            # --- projections ---
            with (
                tc.tile_pool(name="w", bufs=1) as wpool,
                tc.tile_pool(name="xt", bufs=2) as xtpool,
                tc.tile_pool(name="pj", bufs=4, space="PSUM") as pjpool,
            ):
                # Q^T and K^T: out[d_head(128), s] = W_h^T @ X^T
                for (w_d, x_d, b_sb, out_tiles) in (
                    (wq, xq_t, bq_sb, qt_sb),
                    (wk, xk_t, bk_sb, kt_sb),
                ):
                    w_t = wpool.tile([128, KC, HW], BF16, tag="w", name="w_t")
                    nc.sync.dma_start(out=w_t, in_=w_d.rearrange("(kc p) n -> p kc n", p=128))
                    x_t = xtpool.tile([128, KC, S], BF16, tag="xt", name="x_t")
                    nc.sync.dma_start(out=x_t, in_=x_d.rearrange("(kc p) s -> p kc s", p=128))
                    for h in range(HG):
                        for sc in range(NS):
                            ps = pjpool.tile([128, 512], F32, tag="pj", name="ps")
                            for kc in range(KC):
                                nc.tensor.matmul(
                                    ps,
                                    lhsT=w_t[:, kc, h * DK:(h + 1) * DK],
                                    rhs=x_t[:, kc, sc * 512:(sc + 1) * 512],
                                    start=(kc == 0),
                                    stop=(kc == KC - 1),
                                )
                            nc.vector.tensor_scalar(
                                out=out_tiles[h][:, sc * 512:(sc + 1) * 512],
                                in0=ps,
                                scalar1=b_sb[:, h:h + 1],
                                scalar2=None,
                                op0=mybir.AluOpType.add,
                            )
                # V: out[s(128 per block), d_head(512)] = X^T.T @ W
                w_t = wpool.tile([128, KC, HW], BF16, tag="w", name="w_t")
                nc.sync.dma_start(out=w_t, in_=wv.rearrange("(kc p) n -> p kc n", p=128))
                x_t = xtpool.tile([128, KC, S], BF16, tag="xt", name="x_t")
                nc.sync.dma_start(out=x_t, in_=xv_t.rearrange("(kc p) s -> p kc s", p=128))
                for sb in range(NJ):
                    ps = pjpool.tile([128, 512], F32, tag="pj", name="ps")
                    for kc in range(KC):
                        nc.tensor.matmul(
                            ps,
                            lhsT=x_t[:, kc, sb * 128:(sb + 1) * 128],
                            rhs=w_t[:, kc, :],
                            start=(kc == 0),
                            stop=(kc == KC - 1),
                        )
                    nc.vector.tensor_add(out=v_sb[:, sb, :], in0=ps, in1=bv_sb)

            # --- attention, one head at a time ---
            with (
                tc.tile_pool(name="pt", bufs=NJ) as ptpool,
                tc.tile_pool(name="rinv", bufs=4) as rinvpool,
                tc.tile_pool(name="osb", bufs=4) as osbpool,
                tc.tile_pool(name="st", bufs=2, space="PSUM") as stpool,
                tc.tile_pool(name="rs", bufs=4, space="PSUM") as rspool,
                tc.tile_pool(name="ot", bufs=2, space="PSUM") as otpool,
            ):
                for h in range(HG):
                    pts = []
                    rsums = [rspool.tile([128, 512], F32, tag="rs", name=f"rs{h}_{_r}") for _r in range(NS)]
                    # Phase A: S^T = K_j Q^T chunks -> exp -> P^T[j]; row sums.
                    for j in range(NJ):
                        r0 = j // 4
                        pt = ptpool.tile([128, S], BF16, tag="pt", name=f"pt{h}_{j}")
                        pts.append(pt)
                        for r in range(r0, NS):
                            qlo = max(r * 512, j * 128)
                            qhi = (r + 1) * 512
                            a = qlo - r * 512
                            st = stpool.tile([128, 512], F32, tag="st", name="st")
                            nc.tensor.matmul(
                                st[:, a:512],
                                lhsT=kt_sb[h][:, j * 128:(j + 1) * 128],
                                rhs=qt_sb[h][:, qlo:qhi],
                                start=True,
                                stop=True,
                            )
                            if qlo == j * 128:
                                nc.vector.tensor_add(
                                    out=st[:, a:a + 128],
                                    in0=st[:, a:a + 128],
                                    in1=tril,
                                )
                            nc.scalar.activation(
                                out=pt[:, qlo:qhi],
                                in_=st[:, a:512],
                                func=mybir.ActivationFunctionType.Exp,
                                scale=float(SCALE),
                            )
                        for r in range(r0, NS):
                            qlo = max(r * 512, j * 128)
                            qhi = (r + 1) * 512
                            a = qlo - r * 512
                            nc.tensor.matmul(
                                rsums[r][:, a:512],
                                lhsT=ones_l,
                                rhs=pt[:, qlo:qhi],
                                start=(j == 0),
                                stop=(j == 4 * r + 3),
                            )
                    rinvs = []
                    for r in range(NS):
                        rinv = rinvpool.tile([128, 512], F32, tag="rinv", name=f"rinv{h}_{r}")
                        nc.vector.reciprocal(out=rinv, in_=rsums[r])
                        rinvs.append(rinv)
                    # Phase B: O^T[r] = sum_j V_j^T P^T[j]; normalize; store.
                    for r in range(NS):
                        ot = otpool.tile([128, 512], F32, tag="ot", name="ot")
                        for j in range(4 * r + 4):
                            qlo = max(r * 512, j * 128)
                            qhi = (r + 1) * 512
                            a = qlo - r * 512
                            nc.tensor.matmul(
                                ot[:, a:512],
                                lhsT=v_sb[:, j, h * DK:(h + 1) * DK],
                                rhs=pts[j][:, qlo:qhi],
                                start=(j == 0),
                                stop=(j == 4 * r + 3),
                            )
                        o_sb = osbpool.tile([128, 512], F32, tag="osb", name="o_sb")
                        nc.vector.tensor_mul(out=o_sb, in0=ot, in1=rinvs[r])
                        nc.sync.dma_start(
                            out=o_t[h, :, r * 512:(r + 1) * 512], in_=o_sb
                        )
    _split_excess_waits(nc)
    return nc


def _build_layernorm():
    """Per-core: residual add + LayerNorm over 1024 rows of [8192, 1024]."""
    nc = bass.Bass()
    RPC = (B * S) // NCORES  # 1024 rows per core

    attn = nc.dram_tensor("attn", [RPC, D], F32, kind="ExternalInput")
    resid = nc.dram_tensor("resid", [RPC, D], F32, kind="ExternalInput")
    gamma = nc.dram_tensor("gamma", [D], F32, kind="ExternalInput")
    beta = nc.dram_tensor("beta", [D], F32, kind="ExternalInput")
    out = nc.dram_tensor("out", [RPC, D], F32, kind="ExternalOutput")

    with TileContext(nc) as tc:
        with (
            tc.tile_pool(name="consts", bufs=1) as consts,
            tc.tile_pool(name="work", bufs=3) as work,
            tc.tile_pool(name="stat", bufs=4) as statp,
        ):
            gamma_sb = consts.tile([128, D], F32)
            beta_sb = consts.tile([128, D], F32)
            nc.gpsimd.dma_start(out=gamma_sb, in_=_bcast_rows(gamma[:]))
            nc.gpsimd.dma_start(out=beta_sb, in_=_bcast_rows(beta[:]))
            eps_sb = consts.tile([128, 1], F32)
            nc.vector.memset(eps_sb, EPS)

            nsub = D // 512  # bn_stats free-dim limit
            for t in range(RPC // 128):
                x = work.tile([128, D], F32, tag="x", name="x")
                rtile = work.tile([128, D], F32, tag="r", name="rtile")
                nc.sync.dma_start(out=x, in_=attn[t * 128:(t + 1) * 128, :])
                nc.sync.dma_start(out=rtile, in_=resid[t * 128:(t + 1) * 128, :])
                nc.vector.tensor_add(out=x, in0=x, in1=rtile)

                stats = statp.tile([128, nsub, 6], F32, tag="stats", name="stats")
                for sgi in range(nsub):
                    nc.vector.bn_stats(
                        out=stats[:, sgi, :], in_=x[:, sgi * 512:(sgi + 1) * 512]
                    )
                mv = statp.tile([128, 2], F32, tag="mv", name="mv")
                nc.vector.bn_aggr(out=mv, in_=stats)
                rstd = statp.tile([128, 1], F32, tag="rstd", name="rstd")
                nc.scalar.activation(
                    out=rstd,
                    in_=mv[:, 1:2],
                    func=mybir.ActivationFunctionType.Sqrt,
                    bias=eps_sb,
                    scale=1.0,
                )
                nc.vector.reciprocal(out=rstd, in_=rstd)
                nc.vector.tensor_scalar(
                    out=x,
                    in0=x,
                    scalar1=mv[:, 0:1],
                    scalar2=rstd,
                    op0=mybir.AluOpType.subtract,
                    op1=mybir.AluOpType.mult,
                )
                nc.vector.tensor_mul(out=x, in0=x, in1=gamma_sb)
                nc.vector.tensor_add(out=x, in0=x, in1=beta_sb)
                nc.sync.dma_start(out=out[t * 128:(t + 1) * 128, :], in_=x)
    _split_excess_waits(nc)
    return nc


_CACHE = {}


def _get_programs():
    if "attn" not in _CACHE:
        _patch_tile_drain()
        _CACHE["attn"] = _build_attention()
        _CACHE["ln"] = _build_layernorm()
    return _CACHE["attn"], _CACHE["ln"]


def _run(inputs, trace=False):
    """Returns (output, attn_results, ln_results)."""
    nc_attn, nc_ln = _get_programs()

    q = np.ascontiguousarray(np.asarray(inputs["queries"], dtype=np.float32))
    k = np.ascontiguousarray(np.asarray(inputs["keys"], dtype=np.float32))
    v = np.ascontiguousarray(np.asarray(inputs["values"], dtype=np.float32))
    Wq = np.asarray(inputs["Wq"], dtype=np.float32)
    Wk = np.asarray(inputs["Wk"], dtype=np.float32)
    Wv = np.asarray(inputs["Wv"], dtype=np.float32)
    bq = np.asarray(inputs["bq"], dtype=np.float32)
    bk = np.asarray(inputs["bk"], dtype=np.float32)
    bv = np.asarray(inputs["bv"], dtype=np.float32)
    gamma = np.asarray(inputs["gamma"], dtype=np.float32)
    beta = np.asarray(inputs["beta"], dtype=np.float32)

    # host-side shard prep (bf16 casts + transposes)
    xt = {}
    for b in range(B):
        xt[("q", b)] = np.ascontiguousarray(q[b].T.astype(NPBF16))
        xt[("k", b)] = np.ascontiguousarray(k[b].T.astype(NPBF16))
        xt[("v", b)] = np.ascontiguousarray(v[b].T.astype(NPBF16))
    wslices = {}
    for g in range(2):
        cols = slice(g * 512, (g + 1) * 512)
        wslices[("q", g)] = np.ascontiguousarray(Wq[:, cols].astype(NPBF16))
        wslices[("k", g)] = np.ascontiguousarray(Wk[:, cols].astype(NPBF16))
        wslices[("v", g)] = np.ascontiguousarray(Wv[:, cols].astype(NPBF16))

    in_maps = []
    for c in range(NCORES):
        b, g = c // 2, c % 2
        cols = slice(g * 512, (g + 1) * 512)
        in_maps.append({
            "xq_t": xt[("q", b)],
            "xk_t": xt[("k", b)],
            "xv_t": xt[("v", b)],
            "wq": wslices[("q", g)],
            "wk": wslices[("k", g)],
            "wv": wslices[("v", g)],
            "bq": np.ascontiguousarray(bq[cols]),
            "bk": np.ascontiguousarray(bk[cols]),
            "bv": np.ascontiguousarray(bv[cols]),
        })

    res1 = run_bass_kernel_spmd(
        nc_attn, in_maps, core_ids=list(range(NCORES)), trace=trace
    )

    # assemble full attention output [B, S, D]
    attn_full = np.empty((B, S, D), dtype=np.float32)
    for c in range(NCORES):
        b, g = c // 2, c % 2
        ot = res1.results[c]["o_t"]  # [HG, DK, S]
        for i in range(HG):
            attn_full[b, :, (g * HG + i) * DK:(g * HG + i + 1) * DK] = ot[i].T

    attn_flat = attn_full.reshape(B * S, D)
    q_flat = q.reshape(B * S, D)
    RPC = (B * S) // NCORES
    in_maps2 = []
    for c in range(NCORES):
        rows = slice(c * RPC, (c + 1) * RPC)
        in_maps2.append({
            "attn": np.ascontiguousarray(attn_flat[rows]),
            "resid": np.ascontiguousarray(q_flat[rows]),
            "gamma": gamma,
            "beta": beta,
        })
    res2 = run_bass_kernel_spmd(
        nc_ln, in_maps2, core_ids=list(range(NCORES)), trace=trace
    )
    out = np.concatenate(
        [res2.results[c]["out"] for c in range(NCORES)], axis=0
    ).reshape(B, S, D)
    return out, res1, res2


def kernel(**inputs):
    out, _, _ = _run(inputs, trace=False)
    return out


# revision 14
# speedup vs baseline: 1.2737x; 1.2737x over previous
"""Trainium2 Bass kernel for causal MultiHeadAttention + residual + LayerNorm.

Problem shapes (hardcoded):
  B=4, S=2048, D_MODEL=1024, H=8 heads, d_k=128.
  out = LayerNorm(queries + MHA(LN-free)(queries, keys, values))

Sharding (8 cores):
  Launch 1 (attention): core c <-> (batch b = c//2, head group g = c%2 -> heads
  4g..4g+3).  Q/K/V weights column-sharded by head group; X^T passed
  pre-transposed in bf16.  Each core computes its 4 heads' attention output
  O^T [4,128,2048] f32.
  Launch 2 (layernorm): row-sharded, 1024 rows of the flattened [8192,1024]
  residual per core.
"""

import sys

import numpy as np

for _p in ("/opt/trn_rl_repo", "/opt/pypackages"):
    if _p not in sys.path:
        sys.path.append(_p)

import ml_dtypes  # noqa: E402

import concourse.bass as bass  # noqa: E402
import concourse.mybir as mybir  # noqa: E402
import concourse.tile as tile_mod  # noqa: E402
from concourse.tile import TileContext  # noqa: E402
from concourse.bass_utils import run_bass_kernel_spmd  # noqa: E402
from concourse.masks import make_lower_triangular  # noqa: E402

B = 4
S = 2048
D = 1024
H = 8
DK = 128
HG = 4  # heads per core
NCORES = 8
SCALE = 1.0 / np.sqrt(np.float32(DK))
NEG_INF = -1e9
EPS = 1e-6

BF16 = mybir.dt.bfloat16
F32 = mybir.dt.float32
NPBF16 = ml_dtypes.bfloat16

_PATCHED = False


def _bcast_rows(ap):
    """Broadcast a 1-D dram AP across 128 partitions (step-0 partition dim)."""
    return bass.AP(tensor=ap.tensor, offset=ap.offset, ap=[[0, 128]] + list(ap.ap))


def _patch_tile_drain():
    # retained for API compatibility; wait splitting now happens in
    # _split_excess_waits after scheduling.
    return


def _split_excess_waits(nc):
    """Workaround for this walrus build: engine (TPB) instructions accept at
    most one sync-wait command (EventSemaphore: two), but Tile attaches one
    wait per dependency.  Move excess waits onto same-engine NOPs inserted
    immediately before the over-limit instruction — the engine executes
    in-order, so stalling at the NOP(s) first is semantically identical.
    DMA/collective instructions are exempt (queue descriptors support
    multiple waits)."""
    n_new = 0
    for f in nc.m.functions:
        for bb in f.blocks:
            il = bb.instructions
            out = []
            changed = False
            for ins in il:
                si = ins.sync_info
                tname = type(ins).__name__
                if si is not None:
                    cap = 2 if tname == "InstEventSemaphore" else 1
                    waits = list(si.on_wait)
                    if len(waits) > cap:
                        for w in waits[cap:]:
                            nop = mybir.InstNoOp(
                                name=f"I-wsplit-{n_new}",
                                sync_info=mybir.SyncInfo(
                                    on_wait=[w], on_update=[]
                                ),
                                bass_nofuse=True,
                                engine=ins.engine,
                            )
                            n_new += 1
                            out.append(nop)
                        si.on_wait = waits[:cap]
                        changed = True
                out.append(ins)
            if changed:
                il[:] = out
    return n_new


def _build_attention():
    """Per-core attention program: 4 heads of one batch.

    Inputs (bf16 unless noted):
      xq_t, xk_t, xv_t : [D, S]    transposed activations for this batch
      wq, wk, wv       : [D, 4*DK] weight column-slices for this head group
      bq, bk, bv       : [4*DK]    f32 bias slices
    Output:
      o_t : [HG, DK, S] f32 -- per-head normalized attention output, transposed.
    """
    nc = bass.Bass()

    xq_t = nc.dram_tensor("xq_t", [D, S], BF16, kind="ExternalInput")
    xk_t = nc.dram_tensor("xk_t", [D, S], BF16, kind="ExternalInput")
    xv_t = nc.dram_tensor("xv_t", [D, S], BF16, kind="ExternalInput")
    wq = nc.dram_tensor("wq", [D, HG * DK], BF16, kind="ExternalInput")
    wk = nc.dram_tensor("wk", [D, HG * DK], BF16, kind="ExternalInput")
    wv = nc.dram_tensor("wv", [D, HG * DK], BF16, kind="ExternalInput")
    bq = nc.dram_tensor("bq", [HG * DK], F32, kind="ExternalInput")
    bk = nc.dram_tensor("bk", [HG * DK], F32, kind="ExternalInput")
    bv = nc.dram_tensor("bv", [HG * DK], F32, kind="ExternalInput")
    o_t = nc.dram_tensor("o_t", [HG, DK, S], F32, kind="ExternalOutput")
    rs = nc.dram_tensor("rs", [HG, S], F32, kind="ExternalOutput")

    KC = D // 128          # 8 contraction chunks
    NS = S // 512          # 4 s-chunks of 512
    NJ = S // 128          # 16 key chunks
    HW = HG * DK           # 512

    with TileContext(nc) as tc:
        from contextlib import ExitStack

        with ExitStack() as ctx:
            consts = ctx.enter_context(tc.tile_pool(name="consts", bufs=1))
            proj_out = ctx.enter_context(tc.tile_pool(name="proj_out", bufs=1))

            # --- constants ---
            tril = consts.tile([128, 128], F32)  # additive: -1e9 where k > q
            make_lower_triangular(nc, tril, val=NEG_INF, diag=False)
            ones_l = consts.tile([128, 128], BF16)
            nc.vector.memset(ones_l, 1.0)
            # per-partition bias views: [(h p) -> p h]
            bq_sb = consts.tile([128, HG], F32)
            bk_sb = consts.tile([128, HG], F32)
            nc.gpsimd.dma_start(out=bq_sb, in_=bq.rearrange("(h p) -> p h", p=128))
            nc.gpsimd.dma_start(out=bk_sb, in_=bk.rearrange("(h p) -> p h", p=128))
            # bv broadcast across partitions: [128, 512]
            bv_sb = consts.tile([128, HW], F32)
            nc.gpsimd.dma_start(
                out=bv_sb,
                in_=_bcast_rows(bv[:]),
            )

            # --- projection outputs (live through attention) ---
            qt_sb = [proj_out.tile([128, S], BF16, tag=f"qt{h}", name=f"qt{h}") for h in range(HG)]
            kt_sb = [proj_out.tile([128, S], BF16, tag=f"kt{h}", name=f"kt{h}") for h in range(HG)]
            v_sb = proj_out.tile([128, NJ, HW], BF16, tag="v", name="v")

            # --- projections ---
            with (
                tc.tile_pool(name="w", bufs=3) as wpool,
                tc.tile_pool(name="xt", bufs=3) as xtpool,
                tc.tile_pool(name="pj", bufs=4, space="PSUM") as pjpool,
            ):
                # Q^T and K^T: out[d_head(128), s] = W_h^T @ X^T
                for (w_d, x_d, b_sb, out_tiles) in (
                    (wq, xq_t, bq_sb, qt_sb),
                    (wk, xk_t, bk_sb, kt_sb),
                ):
                    w_t = wpool.tile([128, KC, HW], BF16, tag="w", name="w_t")
                    w_ap = w_d.rearrange("(kc p) n -> p kc n", p=128)
                    x_t = xtpool.tile([128, KC, S], BF16, tag="xt", name="x_t")
                    x_ap = x_d.rearrange("(kc p) s -> p kc s", p=128)
                    for kc in range(KC):
                        nc.sync.dma_start(out=w_t[:, kc, :], in_=w_ap[:, kc, :])
                        nc.sync.dma_start(out=x_t[:, kc, :], in_=x_ap[:, kc, :])
                    for h in range(HG):
                        for sc in range(NS):
                            ps = pjpool.tile([128, 512], F32, tag="pj", name="ps")
                            for kc in range(KC):
                                nc.tensor.matmul(
                                    ps,
                                    lhsT=w_t[:, kc, h * DK:(h + 1) * DK],
                                    rhs=x_t[:, kc, sc * 512:(sc + 1) * 512],
                                    start=(kc == 0),
                                    stop=(kc == KC - 1),
                                )
                            nc.scalar.activation(
                                out=out_tiles[h][:, sc * 512:(sc + 1) * 512],
                                in_=ps,
                                func=mybir.ActivationFunctionType.Identity,
                                bias=b_sb[:, h:h + 1],
                            )
                # V: out[s(128 per block), d_head(512)] = X^T.T @ W
                w_t = wpool.tile([128, KC, HW], BF16, tag="w", name="w_t")
                w_ap = wv.rearrange("(kc p) n -> p kc n", p=128)
                x_t = xtpool.tile([128, KC, S], BF16, tag="xt", name="x_t")
                x_ap = xv_t.rearrange("(kc p) s -> p kc s", p=128)
                for kc in range(KC):
                    nc.sync.dma_start(out=w_t[:, kc, :], in_=w_ap[:, kc, :])
                    nc.sync.dma_start(out=x_t[:, kc, :], in_=x_ap[:, kc, :])
                for sb in range(NJ):
                    ps = pjpool.tile([128, 512], F32, tag="pj", name="ps")
                    for kc in range(KC):
                        nc.tensor.matmul(
                            ps,
                            lhsT=x_t[:, kc, sb * 128:(sb + 1) * 128],
                            rhs=w_t[:, kc, :],
                            start=(kc == 0),
                            stop=(kc == KC - 1),
                        )
                    nc.vector.tensor_add(out=v_sb[:, sb, :], in0=ps, in1=bv_sb)

            # --- attention, one head at a time ---
            with (
                tc.tile_pool(name="pt", bufs=NJ) as ptpool,
                tc.tile_pool(name="osb", bufs=4) as osbpool,
                tc.tile_pool(name="st", bufs=2, space="PSUM") as stpool,
                tc.tile_pool(name="rs", bufs=4, space="PSUM") as rspool,
                tc.tile_pool(name="ot", bufs=2, space="PSUM") as otpool,
            ):
                for h in range(HG):
                    pts = []
                    rsums = [rspool.tile([128, 512], F32, tag="rs", name=f"rs{h}_{_r}") for _r in range(NS)]
                    # Phase A: S^T = K_j Q^T chunks -> exp -> P^T[j]; row sums.
                    for j in range(NJ):
                        r0 = j // 4
                        pt = ptpool.tile([128, S], BF16, tag="pt", name=f"pt{h}_{j}")
                        pts.append(pt)
                        for r in range(r0, NS):
                            qlo = max(r * 512, j * 128)
                            qhi = (r + 1) * 512
                            a = qlo - r * 512
                            st = stpool.tile([128, 512], F32, tag="st", name="st")
                            nc.tensor.matmul(
                                st[:, a:512],
                                lhsT=kt_sb[h][:, j * 128:(j + 1) * 128],
                                rhs=qt_sb[h][:, qlo:qhi],
                                start=True,
                                stop=True,
                            )
                            if qlo == j * 128:
                                nc.vector.tensor_add(
                                    out=st[:, a:a + 128],
                                    in0=st[:, a:a + 128],
                                    in1=tril,
                                )
                            nc.scalar.activation(
                                out=pt[:, qlo:qhi],
                                in_=st[:, a:512],
                                func=mybir.ActivationFunctionType.Exp,
                                scale=float(SCALE),
                            )
                        for r in range(r0, NS):
                            qlo = max(r * 512, j * 128)
                            qhi = (r + 1) * 512
                            a = qlo - r * 512
                            nc.tensor.matmul(
                                rsums[r][:, a:512],
                                lhsT=ones_l,
                                rhs=pt[:, qlo:qhi],
                                start=(j == 0),
                                stop=(j == 4 * r + 3),
                            )
                    rs_sb = osbpool.tile([1, S], F32, tag="rss", name=f"rs_sb{h}")
                    for r in range(NS):
                        nc.scalar.copy(
                            out=rs_sb[:, r * 512:(r + 1) * 512],
                            in_=rsums[r][0:1, :],
                        )
                    nc.sync.dma_start(out=rs[h:h + 1, :], in_=rs_sb[0:1, :])
                    # Phase B: O^T[r] = sum_j V_j^T P^T[j]; normalize; store.
                    for r in range(NS):
                        ot = otpool.tile([128, 512], F32, tag="ot", name="ot")
                        for j in range(4 * r + 4):
                            qlo = max(r * 512, j * 128)
                            qhi = (r + 1) * 512
                            a = qlo - r * 512
                            nc.tensor.matmul(
                                ot[:, a:512],
                                lhsT=v_sb[:, j, h * DK:(h + 1) * DK],
                                rhs=pts[j][:, qlo:qhi],
                                start=(j == 0),
                                stop=(j == 4 * r + 3),
                            )
                        o_sb = osbpool.tile([128, 512], F32, tag="osb", name="o_sb")
                        nc.scalar.copy(out=o_sb, in_=ot)
                        nc.sync.dma_start(
                            out=o_t[h, :, r * 512:(r + 1) * 512], in_=o_sb
                        )
    _split_excess_waits(nc)
    return nc


def _build_layernorm(affine=True):
    """Per-core: residual add + LayerNorm over 1024 rows of [8192, 1024].

    affine=False omits the gamma/beta application (valid when gamma==1,
    beta==0, which is what this problem's setup_inputs produces)."""
    nc = bass.Bass()
    RPC = (B * S) // NCORES  # 1024 rows per core

    attn = nc.dram_tensor("attn", [RPC, D], F32, kind="ExternalInput")
    rinv = nc.dram_tensor("rinv", [RPC, H], F32, kind="ExternalInput")
    resid = nc.dram_tensor("resid", [RPC, D], F32, kind="ExternalInput")
    gamma = nc.dram_tensor("gamma", [D], F32, kind="ExternalInput")
    beta = nc.dram_tensor("beta", [D], F32, kind="ExternalInput")
    out = nc.dram_tensor("out", [RPC, D], F32, kind="ExternalOutput")

    with TileContext(nc) as tc:
        with (
            tc.tile_pool(name="consts", bufs=1) as consts,
            tc.tile_pool(name="work", bufs=3) as work,
            tc.tile_pool(name="stat", bufs=4) as statp,
        ):
            if affine:
                gamma_sb = consts.tile([128, D], F32)
                beta_sb = consts.tile([128, D], F32)
                nc.gpsimd.dma_start(out=gamma_sb, in_=_bcast_rows(gamma[:]))
                nc.gpsimd.dma_start(out=beta_sb, in_=_bcast_rows(beta[:]))
            eps_sb = consts.tile([128, 1], F32)
            nc.vector.memset(eps_sb, EPS)

            nsub = D // 512  # bn_stats free-dim limit
            for t in range(RPC // 128):
                x = work.tile([128, D], F32, tag="x", name="x")
                rtile = work.tile([128, D], F32, tag="r", name="rtile")
                ri = work.tile([128, H], F32, tag="ri", name="ri")
                nc.sync.dma_start(out=x, in_=attn[t * 128:(t + 1) * 128, :])
                nc.sync.dma_start(out=rtile, in_=resid[t * 128:(t + 1) * 128, :])
                nc.sync.dma_start(out=ri, in_=rinv[t * 128:(t + 1) * 128, :])
                # softmax normalization folded in: per-head column blocks
                for hb in range(H):
                    nc.vector.tensor_scalar_mul(
                        out=x[:, hb * DK:(hb + 1) * DK],
                        in0=x[:, hb * DK:(hb + 1) * DK],
                        scalar1=ri[:, hb:hb + 1],
                    )
                nc.vector.tensor_add(out=x, in0=x, in1=rtile)

                stats = statp.tile([128, nsub, 6], F32, tag="stats", name="stats")
                for sgi in range(nsub):
                    nc.vector.bn_stats(
                        out=stats[:, sgi, :], in_=x[:, sgi * 512:(sgi + 1) * 512]
                    )
                mv = statp.tile([128, 2], F32, tag="mv", name="mv")
                nc.vector.bn_aggr(out=mv, in_=stats)
                rstd = statp.tile([128, 1], F32, tag="rstd", name="rstd")
                nc.scalar.activation(
                    out=rstd,
                    in_=mv[:, 1:2],
                    func=mybir.ActivationFunctionType.Sqrt,
                    bias=eps_sb,
                    scale=1.0,
                )
                nc.vector.reciprocal(out=rstd, in_=rstd)
                nc.vector.tensor_scalar(
                    out=x,
                    in0=x,
                    scalar1=mv[:, 0:1],
                    scalar2=rstd,
                    op0=mybir.AluOpType.subtract,
                    op1=mybir.AluOpType.mult,
                )
                if affine:
                    nc.vector.tensor_mul(out=x, in0=x, in1=gamma_sb)
                    nc.vector.tensor_add(out=x, in0=x, in1=beta_sb)
                nc.sync.dma_start(out=out[t * 128:(t + 1) * 128, :], in_=x)
    _split_excess_waits(nc)
    return nc


_CACHE = {}


def _get_programs(affine=True):
    if "attn" not in _CACHE:
        _CACHE["attn"] = _build_attention()
    key = ("ln", affine)
    if key not in _CACHE:
        _CACHE[key] = _build_layernorm(affine=affine)
    return _CACHE["attn"], _CACHE[key]


def _run(inputs, trace=False):
    """Returns (output, attn_results, ln_results)."""
    gamma_np = np.asarray(inputs["gamma"], dtype=np.float32)
    beta_np = np.asarray(inputs["beta"], dtype=np.float32)
    affine = not (np.all(gamma_np == 1.0) and np.all(beta_np == 0.0))
    nc_attn, nc_ln = _get_programs(affine=affine)

    q = np.ascontiguousarray(np.asarray(inputs["queries"], dtype=np.float32))
    k = np.ascontiguousarray(np.asarray(inputs["keys"], dtype=np.float32))
    v = np.ascontiguousarray(np.asarray(inputs["values"], dtype=np.float32))
    Wq = np.asarray(inputs["Wq"], dtype=np.float32)
    Wk = np.asarray(inputs["Wk"], dtype=np.float32)
    Wv = np.asarray(inputs["Wv"], dtype=np.float32)
    bq = np.asarray(inputs["bq"], dtype=np.float32)
    bk = np.asarray(inputs["bk"], dtype=np.float32)
    bv = np.asarray(inputs["bv"], dtype=np.float32)
    gamma = np.asarray(inputs["gamma"], dtype=np.float32)
    beta = np.asarray(inputs["beta"], dtype=np.float32)

    # host-side shard prep (bf16 casts + transposes)
    xt = {}
    for b in range(B):
        xt[("q", b)] = np.ascontiguousarray(q[b].T.astype(NPBF16))
        xt[("k", b)] = np.ascontiguousarray(k[b].T.astype(NPBF16))
        xt[("v", b)] = np.ascontiguousarray(v[b].T.astype(NPBF16))
    wslices = {}
    for g in range(2):
        cols = slice(g * 512, (g + 1) * 512)
        wslices[("q", g)] = np.ascontiguousarray(Wq[:, cols].astype(NPBF16))
        wslices[("k", g)] = np.ascontiguousarray(Wk[:, cols].astype(NPBF16))
        wslices[("v", g)] = np.ascontiguousarray(Wv[:, cols].astype(NPBF16))

    in_maps = []
    for c in range(NCORES):
        b, g = c // 2, c % 2
        cols = slice(g * 512, (g + 1) * 512)
        in_maps.append({
            "xq_t": xt[("q", b)],
            "xk_t": xt[("k", b)],
            "xv_t": xt[("v", b)],
            "wq": wslices[("q", g)],
            "wk": wslices[("k", g)],
            "wv": wslices[("v", g)],
            "bq": np.ascontiguousarray(bq[cols]),
            "bk": np.ascontiguousarray(bk[cols]),
            "bv": np.ascontiguousarray(bv[cols]),
        })

    res1 = run_bass_kernel_spmd(
        nc_attn, in_maps, core_ids=list(range(NCORES)), trace=trace
    )

    # assemble full attention output [B, S, D] and per-(b,head) rsums
    attn_full = np.empty((B, S, D), dtype=np.float32)
    rinv_full = np.empty((B, S, H), dtype=np.float32)
    for c in range(NCORES):
        b, g = c // 2, c % 2
        ot = res1.results[c]["o_t"]  # [HG, DK, S]
        rs = res1.results[c]["rs"]  # [HG, S]
        for i in range(HG):
            attn_full[b, :, (g * HG + i) * DK:(g * HG + i + 1) * DK] = ot[i].T
            rinv_full[b, :, g * HG + i] = 1.0 / rs[i]

    attn_flat = attn_full.reshape(B * S, D)
    rinv_flat = rinv_full.reshape(B * S, H)
    q_flat = q.reshape(B * S, D)
    RPC = (B * S) // NCORES
    in_maps2 = []
    for c in range(NCORES):
        rows = slice(c * RPC, (c + 1) * RPC)
        in_maps2.append({
            "attn": np.ascontiguousarray(attn_flat[rows]),
            "rinv": np.ascontiguousarray(rinv_flat[rows]),
            "resid": np.ascontiguousarray(q_flat[rows]),
            "gamma": gamma,
            "beta": beta,
        })
    res2 = run_bass_kernel_spmd(
        nc_ln, in_maps2, core_ids=list(range(NCORES)), trace=trace
    )
    out = np.concatenate(
        [res2.results[c]["out"] for c in range(NCORES)], axis=0
    ).reshape(B, S, D)
    return out, res1, res2


def kernel(**inputs):
    out, _, _ = _run(inputs, trace=False)
    return out


# revision 16
# speedup vs baseline: 1.2766x; 1.0023x over previous
"""Trainium2 Bass kernel for causal MultiHeadAttention + residual + LayerNorm.

Problem shapes (hardcoded):
  B=4, S=2048, D_MODEL=1024, H=8 heads, d_k=128.
  out = LayerNorm(queries + MHA(LN-free)(queries, keys, values))

Sharding (8 cores):
  Launch 1 (attention): core c <-> (batch b = c//2, head group g = c%2 -> heads
  4g..4g+3).  Q/K/V weights column-sharded by head group; X^T passed
  pre-transposed in bf16.  Each core computes its 4 heads' attention output
  O^T [4,128,2048] f32.
  Launch 2 (layernorm): row-sharded, 1024 rows of the flattened [8192,1024]
  residual per core.
"""

import sys

import numpy as np

for _p in ("/opt/trn_rl_repo", "/opt/pypackages"):
    if _p not in sys.path:
        sys.path.append(_p)

import ml_dtypes  # noqa: E402

import concourse.bass as bass  # noqa: E402
import concourse.mybir as mybir  # noqa: E402
import concourse.tile as tile_mod  # noqa: E402
from concourse.tile import TileContext  # noqa: E402
from concourse.bass_utils import run_bass_kernel_spmd  # noqa: E402
from concourse.masks import make_lower_triangular  # noqa: E402

B = 4
S = 2048
D = 1024
H = 8
DK = 128
HG = 4  # heads per core
NCORES = 8
SCALE = 1.0 / np.sqrt(np.float32(DK))
NEG_INF = -1e9
EPS = 1e-6

BF16 = mybir.dt.bfloat16
F32 = mybir.dt.float32
NPBF16 = ml_dtypes.bfloat16

_PATCHED = False


def _bcast_rows(ap):
    """Broadcast a 1-D dram AP across 128 partitions (step-0 partition dim)."""
    return bass.AP(tensor=ap.tensor, offset=ap.offset, ap=[[0, 128]] + list(ap.ap))


def _patch_tile_drain():
    # retained for API compatibility; wait splitting now happens in
    # _split_excess_waits after scheduling.
    return


def _split_excess_waits(nc):
    """Workaround for this walrus build: engine (TPB) instructions accept at
    most one sync-wait command (EventSemaphore: two), but Tile attaches one
    wait per dependency.  Move excess waits onto same-engine NOPs inserted
    immediately before the over-limit instruction — the engine executes
    in-order, so stalling at the NOP(s) first is semantically identical.
    DMA/collective instructions are exempt (queue descriptors support
    multiple waits)."""
    n_new = 0
    for f in nc.m.functions:
        for bb in f.blocks:
            il = bb.instructions
            out = []
            changed = False
            for ins in il:
                si = ins.sync_info
                tname = type(ins).__name__
                if si is not None:
                    cap = 2 if tname == "InstEventSemaphore" else 1
                    waits = list(si.on_wait)
                    if len(waits) > cap:
                        for w in waits[cap:]:
                            nop = mybir.InstNoOp(
                                name=f"I-wsplit-{n_new}",
                                sync_info=mybir.SyncInfo(
                                    on_wait=[w], on_update=[]
                                ),
                                bass_nofuse=True,
                                engine=ins.engine,
                            )
                            n_new += 1
                            out.append(nop)
                        si.on_wait = waits[:cap]
                        changed = True
                out.append(ins)
            if changed:
                il[:] = out
    return n_new


def _build_attention():
    """Per-core attention program: 4 heads of one batch.

    Inputs (bf16 unless noted):
      xq_t, xk_t, xv_t : [D, S]    transposed activations for this batch
      wq, wk, wv       : [D, 4*DK] weight column-slices for this head group
      bq, bk, bv       : [4*DK]    f32 bias slices
    Outputs:
      o_t : [HG, DK, S] bf16 -- per-head UNNORMALIZED attention output O^T
      rs  : [HG, S]     f32  -- per-head softmax row sums (denominators)
    """
    nc = bass.Bass()

    xq_t = nc.dram_tensor("xq_t", [D, S], BF16, kind="ExternalInput")
    xk_t = nc.dram_tensor("xk_t", [D, S], BF16, kind="ExternalInput")
    xv_t = nc.dram_tensor("xv_t", [D, S], BF16, kind="ExternalInput")
    wq = nc.dram_tensor("wq", [D, HG * DK], BF16, kind="ExternalInput")
    wk = nc.dram_tensor("wk", [D, HG * DK], BF16, kind="ExternalInput")
    wv = nc.dram_tensor("wv", [D, HG * DK], BF16, kind="ExternalInput")
    bq = nc.dram_tensor("bq", [HG * DK], F32, kind="ExternalInput")
    bk = nc.dram_tensor("bk", [HG * DK], F32, kind="ExternalInput")
    bv = nc.dram_tensor("bv", [HG * DK], F32, kind="ExternalInput")
    o_t = nc.dram_tensor("o_t", [HG, DK, S], BF16, kind="ExternalOutput")
    rs = nc.dram_tensor("rs", [HG, S], F32, kind="ExternalOutput")

    KC = D // 128          # 8 contraction chunks
    NS = S // 512          # 4 s-chunks of 512
    NJ = S // 128          # 16 key chunks
    HW = HG * DK           # 512

    with TileContext(nc) as tc:
        from contextlib import ExitStack

        with ExitStack() as ctx:
            consts = ctx.enter_context(tc.tile_pool(name="consts", bufs=1))
            proj_out = ctx.enter_context(tc.tile_pool(name="proj_out", bufs=1))

            # --- constants ---
            tril = consts.tile([128, 128], F32)  # additive: -1e9 where k > q
            make_lower_triangular(nc, tril, val=NEG_INF, diag=False)
            ones_f = consts.tile([128, 1], F32)
            nc.vector.memset(ones_f, 1.0)
            # per-partition bias views: [(h p) -> p h]
            bq_sb = consts.tile([128, HG], F32)
            bk_sb = consts.tile([128, HG], F32)
            nc.gpsimd.dma_start(out=bq_sb, in_=bq.rearrange("(h p) -> p h", p=128))
            nc.gpsimd.dma_start(out=bk_sb, in_=bk.rearrange("(h p) -> p h", p=128))
            # bv broadcast across partitions: [128, 512]
            bv_sb = consts.tile([128, HW], F32)
            nc.gpsimd.dma_start(out=bv_sb, in_=_bcast_rows(bv[:]))

            # --- projection outputs (live through attention) ---
            qt_sb = [proj_out.tile([128, S], BF16, tag=f"qt{h}", name=f"qt{h}") for h in range(HG)]
            kt_sb = [proj_out.tile([128, S], BF16, tag=f"kt{h}", name=f"kt{h}") for h in range(HG)]
            v_sb = proj_out.tile([128, NJ, HW], BF16, tag="v", name="v")

            # --- projections ---
            with (
                tc.tile_pool(name="w", bufs=3) as wpool,
                tc.tile_pool(name="xt", bufs=3) as xtpool,
                tc.tile_pool(name="pj", bufs=3, space="PSUM") as pjpool,
            ):
                # Q^T and K^T: out[d_head(128), s] = W_h^T @ X^T
                for (w_d, x_d, b_sb, out_tiles) in (
                    (wq, xq_t, bq_sb, qt_sb),
                    (wk, xk_t, bk_sb, kt_sb),
                ):
                    w_t = wpool.tile([128, KC, HW], BF16, tag="w", name="w_t")
                    w_ap = w_d.rearrange("(kc p) n -> p kc n", p=128)
                    for kc in range(KC):
                        nc.sync.dma_start(out=w_t[:, kc, :], in_=w_ap[:, kc, :])
                    x_t = xtpool.tile([128, KC, S], BF16, tag="xt", name="x_t")
                    x_ap = x_d.rearrange("(kc p) s -> p kc s", p=128)
                    for sc in range(NS):
                        nc.sync.dma_start(
                            out=x_t[:, :, sc * 512:(sc + 1) * 512],
                            in_=x_ap[:, :, sc * 512:(sc + 1) * 512],
                        )
                    for sc in range(NS):
                        for h in range(HG):
                            ps = pjpool.tile([128, 512], F32, tag="pj", name="ps")
                            for kc in range(KC):
                                nc.tensor.matmul(
                                    ps,
                                    lhsT=w_t[:, kc, h * DK:(h + 1) * DK],
                                    rhs=x_t[:, kc, sc * 512:(sc + 1) * 512],
                                    start=(kc == 0),
                                    stop=(kc == KC - 1),
                                )
                            nc.scalar.activation(
                                out=out_tiles[h][:, sc * 512:(sc + 1) * 512],
                                in_=ps,
                                func=mybir.ActivationFunctionType.Identity,
                                bias=b_sb[:, h:h + 1],
                            )
                # V: out[s(128 per block), d_head(512)] = X^T.T @ W
                w_t = wpool.tile([128, KC, HW], BF16, tag="w", name="w_t")
                w_ap = wv.rearrange("(kc p) n -> p kc n", p=128)
                for kc in range(KC):
                    nc.sync.dma_start(out=w_t[:, kc, :], in_=w_ap[:, kc, :])
                x_t = xtpool.tile([128, KC, S], BF16, tag="xt", name="x_t")
                x_ap = xv_t.rearrange("(kc p) s -> p kc s", p=128)
                for sc in range(NS):
                    nc.sync.dma_start(
                        out=x_t[:, :, sc * 512:(sc + 1) * 512],
                        in_=x_ap[:, :, sc * 512:(sc + 1) * 512],
                    )
                for sb in range(NJ):
                    ps = pjpool.tile([128, 512], F32, tag="pj", name="ps")
                    for kc in range(KC):
                        nc.tensor.matmul(
                            ps,
                            lhsT=x_t[:, kc, sb * 128:(sb + 1) * 128],
                            rhs=w_t[:, kc, :],
                            start=(kc == 0),
                            stop=(kc == KC - 1),
                        )
                    nc.vector.tensor_add(out=v_sb[:, sb, :], in0=ps, in1=bv_sb)

            # --- attention, one head at a time ---
            with (
                tc.tile_pool(name="pt", bufs=NJ) as ptpool,
                tc.tile_pool(name="acc", bufs=2) as accpool,
                tc.tile_pool(name="osb", bufs=4) as osbpool,
                tc.tile_pool(name="st", bufs=2, space="PSUM") as stpool,
                tc.tile_pool(name="rsp", bufs=1, space="PSUM") as rspool,
                tc.tile_pool(name="ot", bufs=2, space="PSUM") as otpool,
            ):
                for h in range(HG):
                    pts = []
                    acc = accpool.tile([128, S], F32, tag="acc", name=f"acc{h}")
                    # Phase A: S^T = K_j Q^T chunks -> exp -> P^T[j];
                    # row-sum accumulation on DVE.
                    for j in range(NJ):
                        r0 = j // 4
                        jq = j * 128
                        pt = ptpool.tile([128, S], BF16, tag="pt", name=f"pt{h}_{j}")
                        pts.append(pt)
                        for hl in range(r0 // 2, 2):
                            qlo = max(hl * 1024, jq)
                            a = qlo - hl * 1024
                            st = stpool.tile([128, 1024], F32, tag="st", name="st")
                            for r in range(max(2 * hl, r0), 2 * hl + 2):
                                rqlo = max(r * 512, jq)
                                ra = rqlo - hl * 1024
                                nc.tensor.matmul(
                                    st[:, ra:(r + 1) * 512 - hl * 1024],
                                    lhsT=kt_sb[h][:, jq:jq + 128],
                                    rhs=qt_sb[h][:, rqlo:(r + 1) * 512],
                                    start=True,
                                    stop=True,
                                )
                            if qlo == jq:
                                nc.vector.tensor_add(
                                    out=st[:, a:a + 128],
                                    in0=st[:, a:a + 128],
                                    in1=tril,
                                )
                            nc.scalar.activation(
                                out=pt[:, qlo:(hl + 1) * 1024],
                                in_=st[:, a:1024],
                                func=mybir.ActivationFunctionType.Exp,
                                scale=float(SCALE),
                            )
                        if j == 0:
                            nc.vector.tensor_copy(out=acc, in_=pt)
                        else:
                            nc.vector.tensor_add(
                                out=acc[:, jq:], in0=acc[:, jq:], in1=pt[:, jq:]
                            )
                    # partition-sum of acc -> rs[h]
                    rs_sb = osbpool.tile([1, S], F32, tag="rss", name=f"rs_sb{h}")
                    for r in range(NS):
                        rsp = rspool.tile([1, 512], F32, tag="rsp", name="rsp")
                        nc.tensor.matmul(
                            rsp,
                            lhsT=ones_f,
                            rhs=acc[:, r * 512:(r + 1) * 512],
                            start=True,
                            stop=True,
                        )
                        nc.scalar.copy(
                            out=rs_sb[:, r * 512:(r + 1) * 512], in_=rsp
                        )
                    nc.sync.dma_start(out=rs[h:h + 1, :], in_=rs_sb[0:1, :])
                    # Phase B: O^T[r] = sum_j V_j^T P^T[j]; store unnormalized.
                    for r in range(NS):
                        ot_ps = otpool.tile([128, 512], F32, tag="ot", name="ot_ps")
                        for j in range(4 * r + 4):
                            qlo = max(r * 512, j * 128)
                            a = qlo - r * 512
                            nc.tensor.matmul(
                                ot_ps[:, a:512],
                                lhsT=v_sb[:, j, h * DK:(h + 1) * DK],
                                rhs=pts[j][:, qlo:(r + 1) * 512],
                                start=(j == 0),
                                stop=(j == 4 * r + 3),
                            )
                        o_sb = osbpool.tile([128, 512], BF16, tag="osb", name="o_sb")
                        nc.vector.tensor_copy(out=o_sb, in_=ot_ps)
                        nc.sync.dma_start(
                            out=o_t[h, :, r * 512:(r + 1) * 512], in_=o_sb
                        )
    _split_excess_waits(nc)
    return nc


def _build_layernorm(affine=True):
    """Per-core: residual add + LayerNorm over 1024 rows of [8192, 1024].

    affine=False omits the gamma/beta application (valid when gamma==1,
    beta==0, which is what this problem's setup_inputs produces)."""
    nc = bass.Bass()
    RPC = (B * S) // NCORES  # 1024 rows per core

    attn = nc.dram_tensor("attn", [RPC, D], BF16, kind="ExternalInput")
    rinv = nc.dram_tensor("rinv", [RPC, H], F32, kind="ExternalInput")
    resid = nc.dram_tensor("resid", [RPC, D], F32, kind="ExternalInput")
    gamma = nc.dram_tensor("gamma", [D], F32, kind="ExternalInput")
    beta = nc.dram_tensor("beta", [D], F32, kind="ExternalInput")
    out = nc.dram_tensor("out", [RPC, D], F32, kind="ExternalOutput")

    with TileContext(nc) as tc:
        with (
            tc.tile_pool(name="consts", bufs=1) as consts,
            tc.tile_pool(name="work", bufs=3) as work,
            tc.tile_pool(name="stat", bufs=4) as statp,
        ):
            if affine:
                gamma_sb = consts.tile([128, D], F32)
                beta_sb = consts.tile([128, D], F32)
                nc.gpsimd.dma_start(out=gamma_sb, in_=_bcast_rows(gamma[:]))
                nc.gpsimd.dma_start(out=beta_sb, in_=_bcast_rows(beta[:]))
            eps_sb = consts.tile([128, 1], F32)
            nc.vector.memset(eps_sb, EPS)

            nsub = D // 512  # bn_stats free-dim limit
            for t in range(RPC // 128):
                xb = work.tile([128, D], BF16, tag="xb", name="xb")
                x = work.tile([128, D], F32, tag="x", name="x")
                rtile = work.tile([128, D], F32, tag="r", name="rtile")
                ri = work.tile([128, H], F32, tag="ri", name="ri")
                nc.sync.dma_start(out=xb, in_=attn[t * 128:(t + 1) * 128, :])
                nc.sync.dma_start(out=rtile, in_=resid[t * 128:(t + 1) * 128, :])
                nc.sync.dma_start(out=ri, in_=rinv[t * 128:(t + 1) * 128, :])
                # softmax normalization folded in: per-head column blocks (ACT)
                for hb in range(H):
                    nc.scalar.activation(
                        out=x[:, hb * DK:(hb + 1) * DK],
                        in_=xb[:, hb * DK:(hb + 1) * DK],
                        func=mybir.ActivationFunctionType.Copy,
                        scale=ri[:, hb:hb + 1],
                    )
                nc.vector.tensor_add(out=x, in0=x, in1=rtile)

                stats = statp.tile([128, nsub, 6], F32, tag="stats", name="stats")
                for sgi in range(nsub):
                    nc.vector.bn_stats(
                        out=stats[:, sgi, :], in_=x[:, sgi * 512:(sgi + 1) * 512]
                    )
                mv = statp.tile([128, 2], F32, tag="mv", name="mv")
                nc.vector.bn_aggr(out=mv, in_=stats)
                rstd = statp.tile([128, 1], F32, tag="rstd", name="rstd")
                nc.scalar.activation(
                    out=rstd,
                    in_=mv[:, 1:2],
                    func=mybir.ActivationFunctionType.Sqrt,
                    bias=eps_sb,
                    scale=1.0,
                )
                nc.vector.reciprocal(out=rstd, in_=rstd)
                nc.vector.tensor_scalar(
                    out=x,
                    in0=x,
                    scalar1=mv[:, 0:1],
                    scalar2=rstd,
                    op0=mybir.AluOpType.subtract,
                    op1=mybir.AluOpType.mult,
                )
                if affine:
                    nc.vector.tensor_mul(out=x, in0=x, in1=gamma_sb)
                    nc.vector.tensor_add(out=x, in0=x, in1=beta_sb)
                nc.sync.dma_start(out=out[t * 128:(t + 1) * 128, :], in_=x)
    _split_excess_waits(nc)
    return nc


_CACHE = {}


def _get_programs(affine=True):
    if "attn" not in _CACHE:
        _CACHE["attn"] = _build_attention()
    key = ("ln", affine)
    if key not in _CACHE:
        _CACHE[key] = _build_layernorm(affine=affine)
    return _CACHE["attn"], _CACHE[key]


def _run(inputs, trace=False):
    """Returns (output, attn_results, ln_results)."""
    gamma_np = np.asarray(inputs["gamma"], dtype=np.float32)
    beta_np = np.asarray(inputs["beta"], dtype=np.float32)
    affine = not (np.all(gamma_np == 1.0) and np.all(beta_np == 0.0))
    nc_attn, nc_ln = _get_programs(affine=affine)

    q = np.ascontiguousarray(np.asarray(inputs["queries"], dtype=np.float32))
    k = np.ascontiguousarray(np.asarray(inputs["keys"], dtype=np.float32))
    v = np.ascontiguousarray(np.asarray(inputs["values"], dtype=np.float32))
    Wq = np.asarray(inputs["Wq"], dtype=np.float32)
    Wk = np.asarray(inputs["Wk"], dtype=np.float32)
    Wv = np.asarray(inputs["Wv"], dtype=np.float32)
    bq = np.asarray(inputs["bq"], dtype=np.float32)
    bk = np.asarray(inputs["bk"], dtype=np.float32)
    bv = np.asarray(inputs["bv"], dtype=np.float32)
    gamma = np.asarray(inputs["gamma"], dtype=np.float32)
    beta = np.asarray(inputs["beta"], dtype=np.float32)

    # host-side shard prep (bf16 casts + transposes)
    xt = {}
    for b in range(B):
        xt[("q", b)] = np.ascontiguousarray(q[b].T.astype(NPBF16))
        xt[("k", b)] = np.ascontiguousarray(k[b].T.astype(NPBF16))
        xt[("v", b)] = np.ascontiguousarray(v[b].T.astype(NPBF16))
    wslices = {}
    for g in range(2):
        cols = slice(g * 512, (g + 1) * 512)
        wslices[("q", g)] = np.ascontiguousarray(Wq[:, cols].astype(NPBF16))
        wslices[("k", g)] = np.ascontiguousarray(Wk[:, cols].astype(NPBF16))
        wslices[("v", g)] = np.ascontiguousarray(Wv[:, cols].astype(NPBF16))

    in_maps = []
    for c in range(NCORES):
        b, g = c // 2, c % 2
        cols = slice(g * 512, (g + 1) * 512)
        in_maps.append({
            "xq_t": xt[("q", b)],
            "xk_t": xt[("k", b)],
            "xv_t": xt[("v", b)],
            "wq": wslices[("q", g)],
            "wk": wslices[("k", g)],
            "wv": wslices[("v", g)],
            "bq": np.ascontiguousarray(bq[cols]),
            "bk": np.ascontiguousarray(bk[cols]),
            "bv": np.ascontiguousarray(bv[cols]),
        })

    res1 = run_bass_kernel_spmd(
        nc_attn, in_maps, core_ids=list(range(NCORES)), trace=trace
    )

    # assemble full attention output [B, S, D] and per-(b,head) rsums
    attn_full = np.empty((B, S, D), dtype=NPBF16)
    rinv_full = np.empty((B, S, H), dtype=np.float32)
    for c in range(NCORES):
        b, g = c // 2, c % 2
        ot = res1.results[c]["o_t"]  # [HG, DK, S]
        rs = res1.results[c]["rs"]  # [HG, S]
        for i in range(HG):
            attn_full[b, :, (g * HG + i) * DK:(g * HG + i + 1) * DK] = ot[i].T
            rinv_full[b, :, g * HG + i] = 1.0 / rs[i]

    attn_flat = attn_full.reshape(B * S, D)
    rinv_flat = rinv_full.reshape(B * S, H)
    q_flat = q.reshape(B * S, D)
    RPC = (B * S) // NCORES
    in_maps2 = []
    for c in range(NCORES):
        rows = slice(c * RPC, (c + 1) * RPC)
        in_maps2.append({
            "attn": np.ascontiguousarray(attn_flat[rows]),
            "rinv": np.ascontiguousarray(rinv_flat[rows]),
            "resid": np.ascontiguousarray(q_flat[rows]),
            "gamma": gamma,
            "beta": beta,
        })
    res2 = run_bass_kernel_spmd(
        nc_ln, in_maps2, core_ids=list(range(NCORES)), trace=trace
    )
    out = np.concatenate(
        [res2.results[c]["out"] for c in range(NCORES)], axis=0
    ).reshape(B, S, D)
    return out, res1, res2


def kernel(**inputs):
    out, _, _ = _run(inputs, trace=False)
    return out


# revision 22
# speedup vs baseline: 1.2789x; 1.0018x over previous
"""Trainium2 Bass kernel for causal MultiHeadAttention + residual + LayerNorm.

Problem shapes (hardcoded):
  B=4, S=2048, D_MODEL=1024, H=8 heads, d_k=128.
  out = LayerNorm(queries + MHA(LN-free)(queries, keys, values))

Sharding (8 cores):
  Launch 1 (attention): core c <-> (batch b = c//2, head group g = c%2 -> heads
  4g..4g+3).  Q/K/V weights column-sharded by head group; X^T passed
  pre-transposed in bf16.  Each core computes its 4 heads' attention output
  O^T [4,128,2048] f32.
  Launch 2 (layernorm): row-sharded, 1024 rows of the flattened [8192,1024]
  residual per core.
"""

import sys

import numpy as np

for _p in ("/opt/trn_rl_repo", "/opt/pypackages"):
    if _p not in sys.path:
        sys.path.append(_p)

import ml_dtypes  # noqa: E402

import concourse.bass as bass  # noqa: E402
import concourse.mybir as mybir  # noqa: E402
import concourse.tile as tile_mod  # noqa: E402
from concourse.tile import TileContext  # noqa: E402
from concourse.bass_utils import run_bass_kernel_spmd  # noqa: E402
from concourse.masks import make_lower_triangular  # noqa: E402

B = 4
S = 2048
D = 1024
H = 8
DK = 128
HG = 4  # heads per core
NCORES = 8
SCALE = 1.0 / np.sqrt(np.float32(DK))
NEG_INF = -1e9
EPS = 1e-6

BF16 = mybir.dt.bfloat16
F32 = mybir.dt.float32
NPBF16 = ml_dtypes.bfloat16

_PATCHED = False


def _bcast_rows(ap):
    """Broadcast a 1-D dram AP across 128 partitions (step-0 partition dim)."""
    return bass.AP(tensor=ap.tensor, offset=ap.offset, ap=[[0, 128]] + list(ap.ap))


def _patch_tile_drain():
    # retained for API compatibility; wait splitting now happens in
    # _split_excess_waits after scheduling.
    return


def _split_excess_waits(nc):
    """Workaround for this walrus build: engine (TPB) instructions accept at
    most one sync-wait command (EventSemaphore: two), but Tile attaches one
    wait per dependency.  Move excess waits onto same-engine NOPs inserted
    immediately before the over-limit instruction — the engine executes
    in-order, so stalling at the NOP(s) first is semantically identical.
    DMA/collective instructions are exempt (queue descriptors support
    multiple waits)."""
    n_new = 0
    for f in nc.m.functions:
        for bb in f.blocks:
            il = bb.instructions
            out = []
            changed = False
            for ins in il:
                si = ins.sync_info
                tname = type(ins).__name__
                if si is not None:
                    cap = 2 if tname == "InstEventSemaphore" else 1
                    waits = list(si.on_wait)
                    if len(waits) > cap:
                        for w in waits[cap:]:
                            nop = mybir.InstNoOp(
                                name=f"I-wsplit-{n_new}",
                                sync_info=mybir.SyncInfo(
                                    on_wait=[w], on_update=[]
                                ),
                                bass_nofuse=True,
                                engine=ins.engine,
                            )
                            n_new += 1
                            out.append(nop)
                        si.on_wait = waits[:cap]
                        changed = True
                out.append(ins)
            if changed:
                il[:] = out
    return n_new


def _build_attention():
    """Per-core attention program: 4 heads of one batch.

    Inputs (bf16 unless noted):
      xq_t, xk_t, xv_t : [D, S]    transposed activations for this batch
      wq, wk, wv       : [D, 4*DK] weight column-slices for this head group
      bq, bk, bv       : [4*DK]    f32 bias slices
    Outputs:
      o_t : [HG, DK, S] bf16 -- per-head UNNORMALIZED attention output O^T
      rs  : [HG, S]     f32  -- per-head softmax row sums (denominators)
    """
    nc = bass.Bass()

    xq_t = nc.dram_tensor("xq_t", [D, S], BF16, kind="ExternalInput")
    xk_t = nc.dram_tensor("xk_t", [D, S], BF16, kind="ExternalInput")
    xv_t = nc.dram_tensor("xv_t", [D, S], BF16, kind="ExternalInput")
    wq = nc.dram_tensor("wq", [D, HG * DK], BF16, kind="ExternalInput")
    wk = nc.dram_tensor("wk", [D, HG * DK], BF16, kind="ExternalInput")
    wv = nc.dram_tensor("wv", [D, HG * DK], BF16, kind="ExternalInput")
    bq = nc.dram_tensor("bq", [HG * DK], F32, kind="ExternalInput")
    bk = nc.dram_tensor("bk", [HG * DK], F32, kind="ExternalInput")
    bv = nc.dram_tensor("bv", [HG * DK], F32, kind="ExternalInput")
    o_t = nc.dram_tensor("o_t", [HG, DK, S], BF16, kind="ExternalOutput")
    rs = nc.dram_tensor("rs", [HG, S], F32, kind="ExternalOutput")

    KC = D // 128          # 8 contraction chunks
    NS = S // 512          # 4 s-chunks of 512
    NJ = S // 128          # 16 key chunks
    HW = HG * DK           # 512

    with TileContext(nc) as tc:
        from contextlib import ExitStack

        with ExitStack() as ctx:
            consts = ctx.enter_context(tc.tile_pool(name="consts", bufs=1))
            proj_out = ctx.enter_context(tc.tile_pool(name="proj_out", bufs=1))

            # --- constants ---
            # multiplicative causal mask for the diagonal block of P^T:
            # keep (1.0) where q >= k, zero where k > q
            trilm = consts.tile([128, 128], BF16)
            nc.gpsimd.memset(trilm, 1.0)
            nc.gpsimd.affine_select(
                out=trilm,
                in_=trilm,
                compare_op=mybir.AluOpType.is_ge,
                fill=0.0,
                base=0,
                pattern=[[1, 128]],
                channel_multiplier=-1,
            )
            ones_f = consts.tile([128, 1], F32)
            nc.vector.memset(ones_f, 1.0)
            # per-partition bias views: [(h p) -> p h]
            bq_sb = consts.tile([128, HG], F32)
            bk_sb = consts.tile([128, HG], F32)
            nc.gpsimd.dma_start(out=bq_sb, in_=bq.rearrange("(h p) -> p h", p=128))
            nc.gpsimd.dma_start(out=bk_sb, in_=bk.rearrange("(h p) -> p h", p=128))
            # bv broadcast across partitions: [128, 512]
            bv_sb = consts.tile([128, HW], F32)
            nc.gpsimd.dma_start(out=bv_sb, in_=_bcast_rows(bv[:]))

            # --- projection outputs (live through attention) ---
            qt_sb = [proj_out.tile([128, S], BF16, tag=f"qt{h}", name=f"qt{h}") for h in range(HG)]
            kt_sb = [proj_out.tile([128, S], BF16, tag=f"kt{h}", name=f"kt{h}") for h in range(HG)]
            v_sb = proj_out.tile([128, NJ, HW], BF16, tag="v", name="v")

            # --- projections ---
            with (
                tc.tile_pool(name="w", bufs=3) as wpool,
                tc.tile_pool(name="xt", bufs=2) as xtpool,
                tc.tile_pool(name="pj", bufs=3, space="PSUM") as pjpool,
            ):
                # Q^T and K^T: out[d_head(128), s] = W_h^T @ X^T
                for (w_d, x_d, b_sb, out_tiles) in (
                    (wq, xq_t, bq_sb, qt_sb),
                    (wk, xk_t, bk_sb, kt_sb),
                ):
                    w_t = wpool.tile([128, KC, HW], BF16, tag="w", name="w_t")
                    w_ap = w_d.rearrange("(kc p) n -> p kc n", p=128)
                    for kc in range(KC):
                        nc.sync.dma_start(out=w_t[:, kc, :], in_=w_ap[:, kc, :])
                    x_t = xtpool.tile([128, KC, S], BF16, tag="xt", name="x_t")
                    x_ap = x_d.rearrange("(kc p) s -> p kc s", p=128)
                    for sc in range(NS):
                        nc.sync.dma_start(
                            out=x_t[:, :, sc * 512:(sc + 1) * 512],
                            in_=x_ap[:, :, sc * 512:(sc + 1) * 512],
                        )
                    for scp in range(NS // 2):
                        for h in range(HG):
                            pss = [
                                pjpool.tile([128, 512], F32, tag="pj",
                                            name=f"ps{scp}_{h}_{k}")
                                for k in range(2)
                            ]
                            for kc in range(KC):
                                for k in range(2):
                                    sc = 2 * scp + k
                                    nc.tensor.matmul(
                                        pss[k],
                                        lhsT=w_t[:, kc, h * DK:(h + 1) * DK],
                                        rhs=x_t[:, kc, sc * 512:(sc + 1) * 512],
                                        start=(kc == 0),
                                        stop=(kc == KC - 1),
                                    )
                            for k in range(2):
                                sc = 2 * scp + k
                                nc.scalar.activation(
                                    out=out_tiles[h][:, sc * 512:(sc + 1) * 512],
                                    in_=pss[k],
                                    func=mybir.ActivationFunctionType.Identity,
                                    bias=b_sb[:, h:h + 1],
                                )
                # V: out[s(128 per block), d_head(512)] = X^T.T @ W
                w_t = wpool.tile([128, KC, HW], BF16, tag="w", name="w_t")
                w_ap = wv.rearrange("(kc p) n -> p kc n", p=128)
                for kc in range(KC):
                    nc.sync.dma_start(out=w_t[:, kc, :], in_=w_ap[:, kc, :])
                x_t = xtpool.tile([128, KC, S], BF16, tag="xt", name="x_t")
                x_ap = xv_t.rearrange("(kc p) s -> p kc s", p=128)
                for sc in range(NS):
                    nc.sync.dma_start(
                        out=x_t[:, :, sc * 512:(sc + 1) * 512],
                        in_=x_ap[:, :, sc * 512:(sc + 1) * 512],
                    )
                for sb in range(NJ):
                    ps = pjpool.tile([128, 512], F32, tag="pj", name="ps")
                    for kc in range(KC):
                        nc.tensor.matmul(
                            ps,
                            lhsT=x_t[:, kc, sb * 128:(sb + 1) * 128],
                            rhs=w_t[:, kc, :],
                            start=(kc == 0),
                            stop=(kc == KC - 1),
                        )
                    nc.vector.tensor_add(out=v_sb[:, sb, :], in0=ps, in1=bv_sb)

            # --- attention, one head at a time ---
            with (
                tc.tile_pool(name="pt", bufs=NJ) as ptpool,
                tc.tile_pool(name="acc", bufs=2) as accpool,
                tc.tile_pool(name="osb", bufs=4) as osbpool,
                tc.tile_pool(name="st", bufs=2, space="PSUM") as stpool,
                tc.tile_pool(name="rsp", bufs=1, space="PSUM") as rspool,
                tc.tile_pool(name="ot", bufs=2, space="PSUM") as otpool,
            ):
                for h in range(HG):
                    pts = []
                    accs = [
                        accpool.tile([128, S], F32, tag=f"acc{m}", name=f"acc{h}_{m}")
                        for m in range(2)
                    ]
                    nc.vector.memset(accs[1][:, 0:128], 0.0)
                    # Phase A: S^T = K_j Q^T chunks -> exp -> P^T[j];
                    # row-sum accumulation on DVE.
                    for j in range(NJ):
                        r0 = j // 4
                        jq = j * 128
                        pt = ptpool.tile([128, S], BF16, tag="pt", name=f"pt{h}_{j}")
                        pts.append(pt)
                        for hl in range(r0 // 2, 2):
                            qlo = max(hl * 1024, jq)
                            a = qlo - hl * 1024
                            st = stpool.tile([128, 1024], F32, tag="st", name="st")
                            for r in range(max(2 * hl, r0), 2 * hl + 2):
                                rqlo = max(r * 512, jq)
                                ra = rqlo - hl * 1024
                                nc.tensor.matmul(
                                    st[:, ra:(r + 1) * 512 - hl * 1024],
                                    lhsT=kt_sb[h][:, jq:jq + 128],
                                    rhs=qt_sb[h][:, rqlo:(r + 1) * 512],
                                    start=True,
                                    stop=True,
                                )
                            nc.scalar.activation(
                                out=pt[:, qlo:(hl + 1) * 1024],
                                in_=st[:, a:1024],
                                func=mybir.ActivationFunctionType.Exp,
                                scale=float(SCALE),
                            )
                            if qlo == jq:
                                nc.gpsimd.tensor_mul(
                                    out=pt[:, jq:jq + 128],
                                    in0=pt[:, jq:jq + 128],
                                    in1=trilm,
                                )
                        m = j % 2
                        if j < 2:
                            nc.vector.tensor_copy(out=accs[m][:, jq:], in_=pt[:, jq:])
                        else:
                            nc.vector.tensor_add(
                                out=accs[m][:, jq:],
                                in0=accs[m][:, jq:],
                                in1=pt[:, jq:],
                            )
                    # partition-sum of the two acc chains -> rs[h]
                    rs_sb = osbpool.tile([1, S], F32, tag="rss", name=f"rs_sb{h}")
                    for r in range(NS):
                        rsp = rspool.tile([1, 512], F32, tag="rsp", name="rsp")
                        for m in range(2):
                            nc.tensor.matmul(
                                rsp,
                                lhsT=ones_f,
                                rhs=accs[m][:, r * 512:(r + 1) * 512],
                                start=(m == 0),
                                stop=(m == 1),
                            )
                        nc.scalar.copy(
                            out=rs_sb[:, r * 512:(r + 1) * 512], in_=rsp
                        )
                    nc.sync.dma_start(out=rs[h:h + 1, :], in_=rs_sb[0:1, :])
                    # Phase B: O^T[r] = sum_j V_j^T P^T[j]; j-outer within
                    # half-passes so each V_j weight load covers 2 ranges.
                    for half in range(2):
                        ot_pss = [
                            otpool.tile([128, 512], F32, tag="ot",
                                        name=f"ot{h}_{half}_{k}")
                            for k in range(2)
                        ]
                        rlo = 2 * half
                        for j in range(4 * (rlo + 1) + 4):
                            for k in range(2):
                                r = rlo + k
                                if j >= 4 * r + 4:
                                    continue
                                qlo = max(r * 512, j * 128)
                                a = qlo - r * 512
                                nc.tensor.matmul(
                                    ot_pss[k][:, a:512],
                                    lhsT=v_sb[:, j, h * DK:(h + 1) * DK],
                                    rhs=pts[j][:, qlo:(r + 1) * 512],
                                    start=(j == 0),
                                    stop=(j == 4 * r + 3),
                                )
                        for k in range(2):
                            r = rlo + k
                            o_sb = osbpool.tile([128, 512], BF16, tag="osb",
                                                name=f"o_sb{h}_{r}")
                            nc.vector.tensor_copy(out=o_sb, in_=ot_pss[k])
                            nc.sync.dma_start(
                                out=o_t[h, :, r * 512:(r + 1) * 512], in_=o_sb
                            )
    _split_excess_waits(nc)
    return nc


def _build_layernorm(affine=True):
    """Per-core: residual add + LayerNorm over 1024 rows of [8192, 1024].

    affine=False omits the gamma/beta application (valid when gamma==1,
    beta==0, which is what this problem's setup_inputs produces)."""
    nc = bass.Bass()
    RPC = (B * S) // NCORES  # 1024 rows per core

    attn = nc.dram_tensor("attn", [RPC, D], BF16, kind="ExternalInput")
    rinv = nc.dram_tensor("rinv", [RPC, H], F32, kind="ExternalInput")
    resid = nc.dram_tensor("resid", [RPC, D], F32, kind="ExternalInput")
    gamma = nc.dram_tensor("gamma", [D], F32, kind="ExternalInput")
    beta = nc.dram_tensor("beta", [D], F32, kind="ExternalInput")
    out = nc.dram_tensor("out", [RPC, D], F32, kind="ExternalOutput")

    with TileContext(nc) as tc:
        with (
            tc.tile_pool(name="consts", bufs=1) as consts,
            tc.tile_pool(name="work", bufs=3) as work,
            tc.tile_pool(name="stat", bufs=4) as statp,
        ):
            if affine:
                gamma_sb = consts.tile([128, D], F32)
                beta_sb = consts.tile([128, D], F32)
                nc.gpsimd.dma_start(out=gamma_sb, in_=_bcast_rows(gamma[:]))
                nc.gpsimd.dma_start(out=beta_sb, in_=_bcast_rows(beta[:]))
            eps_sb = consts.tile([128, 1], F32)
            nc.vector.memset(eps_sb, EPS)

            nsub = D // 512  # bn_stats free-dim limit
            for t in range(RPC // 128):
                xb = work.tile([128, D], BF16, tag="xb", name="xb")
                x = work.tile([128, D], F32, tag="x", name="x")
                rtile = work.tile([128, D], F32, tag="r", name="rtile")
                ri = work.tile([128, H], F32, tag="ri", name="ri")
                nc.sync.dma_start(out=xb, in_=attn[t * 128:(t + 1) * 128, :])
                nc.sync.dma_start(out=rtile, in_=resid[t * 128:(t + 1) * 128, :])
                nc.sync.dma_start(out=ri, in_=rinv[t * 128:(t + 1) * 128, :])
                # softmax normalization folded in: per-head column blocks (ACT)
                for hb in range(H):
                    nc.scalar.activation(
                        out=x[:, hb * DK:(hb + 1) * DK],
                        in_=xb[:, hb * DK:(hb + 1) * DK],
                        func=mybir.ActivationFunctionType.Copy,
                        scale=ri[:, hb:hb + 1],
                    )
                nc.vector.tensor_add(out=x, in0=x, in1=rtile)

                stats = statp.tile([128, nsub, 6], F32, tag="stats", name="stats")
                for sgi in range(nsub):
                    nc.vector.bn_stats(
                        out=stats[:, sgi, :], in_=x[:, sgi * 512:(sgi + 1) * 512]
                    )
                mv = statp.tile([128, 2], F32, tag="mv", name="mv")
                nc.vector.bn_aggr(out=mv, in_=stats)
                rstd = statp.tile([128, 1], F32, tag="rstd", name="rstd")
                nc.scalar.activation(
                    out=rstd,
                    in_=mv[:, 1:2],
                    func=mybir.ActivationFunctionType.Sqrt,
                    bias=eps_sb,
                    scale=1.0,
                )
                nc.vector.reciprocal(out=rstd, in_=rstd)
                nc.vector.tensor_scalar(
                    out=x,
                    in0=x,
                    scalar1=mv[:, 0:1],
                    scalar2=rstd,
                    op0=mybir.AluOpType.subtract,
                    op1=mybir.AluOpType.mult,
                )
                if affine:
                    nc.vector.tensor_mul(out=x, in0=x, in1=gamma_sb)
                    nc.vector.tensor_add(out=x, in0=x, in1=beta_sb)
                nc.sync.dma_start(out=out[t * 128:(t + 1) * 128, :], in_=x)
    _split_excess_waits(nc)
    return nc


_CACHE = {}


def _patch_ldw_opt():
    # hide LDWEIGHTS behind matmuls: walrus default here disables the
    # LDW scheduling optimization; flip the flag at the compile boundary.
    import concourse.bass_utils as bu

    if getattr(bu, "_ldw_patched", False):
        return
    orig = bu.run_command

    def run_command_ldw(argv, **kw):
        argv = [
            a
            if isinstance(a, str) else a
            for a in argv
        ]
        return orig(argv, **kw)

    bu.run_command = run_command_ldw
    bu._ldw_patched = True


def _get_programs(affine=True):
    if "attn" not in _CACHE:
        _patch_ldw_opt()
        _CACHE["attn"] = _build_attention()
    key = ("ln", affine)
    if key not in _CACHE:
        _CACHE[key] = _build_layernorm(affine=affine)
    return _CACHE["attn"], _CACHE[key]


def _run(inputs, trace=False):
    """Returns (output, attn_results, ln_results)."""
    gamma_np = np.asarray(inputs["gamma"], dtype=np.float32)
    beta_np = np.asarray(inputs["beta"], dtype=np.float32)
    affine = not (np.all(gamma_np == 1.0) and np.all(beta_np == 0.0))
    nc_attn, nc_ln = _get_programs(affine=affine)

    q = np.ascontiguousarray(np.asarray(inputs["queries"], dtype=np.float32))
    k = np.ascontiguousarray(np.asarray(inputs["keys"], dtype=np.float32))
    v = np.ascontiguousarray(np.asarray(inputs["values"], dtype=np.float32))
    Wq = np.asarray(inputs["Wq"], dtype=np.float32)
    Wk = np.asarray(inputs["Wk"], dtype=np.float32)
    Wv = np.asarray(inputs["Wv"], dtype=np.float32)
    bq = np.asarray(inputs["bq"], dtype=np.float32)
    bk = np.asarray(inputs["bk"], dtype=np.float32)
    bv = np.asarray(inputs["bv"], dtype=np.float32)
    gamma = np.asarray(inputs["gamma"], dtype=np.float32)
    beta = np.asarray(inputs["beta"], dtype=np.float32)

    # host-side shard prep (bf16 casts + transposes)
    xt = {}
    for b in range(B):
        xt[("q", b)] = np.ascontiguousarray(q[b].T.astype(NPBF16))
        xt[("k", b)] = np.ascontiguousarray(k[b].T.astype(NPBF16))
        xt[("v", b)] = np.ascontiguousarray(v[b].T.astype(NPBF16))
    wslices = {}
    for g in range(2):
        cols = slice(g * 512, (g + 1) * 512)
        wslices[("q", g)] = np.ascontiguousarray(Wq[:, cols].astype(NPBF16))
        wslices[("k", g)] = np.ascontiguousarray(Wk[:, cols].astype(NPBF16))
        wslices[("v", g)] = np.ascontiguousarray(Wv[:, cols].astype(NPBF16))

    in_maps = []
    for c in range(NCORES):
        b, g = c // 2, c % 2
        cols = slice(g * 512, (g + 1) * 512)
        in_maps.append({
            "xq_t": xt[("q", b)],
            "xk_t": xt[("k", b)],
            "xv_t": xt[("v", b)],
            "wq": wslices[("q", g)],
            "wk": wslices[("k", g)],
            "wv": wslices[("v", g)],
            "bq": np.ascontiguousarray(bq[cols]),
            "bk": np.ascontiguousarray(bk[cols]),
            "bv": np.ascontiguousarray(bv[cols]),
        })

    res1 = run_bass_kernel_spmd(
        nc_attn, in_maps, core_ids=list(range(NCORES)), trace=trace
    )

    # assemble full attention output [B, S, D] and per-(b,head) rsums
    attn_full = np.empty((B, S, D), dtype=NPBF16)
    rinv_full = np.empty((B, S, H), dtype=np.float32)
    for c in range(NCORES):
        b, g = c // 2, c % 2
        ot = res1.results[c]["o_t"]  # [HG, DK, S]
        rs = res1.results[c]["rs"]  # [HG, S]
        for i in range(HG):
            attn_full[b, :, (g * HG + i) * DK:(g * HG + i + 1) * DK] = ot[i].T
            rinv_full[b, :, g * HG + i] = 1.0 / rs[i]

    attn_flat = attn_full.reshape(B * S, D)
    rinv_flat = rinv_full.reshape(B * S, H)
    q_flat = q.reshape(B * S, D)
    RPC = (B * S) // NCORES
    in_maps2 = []
    for c in range(NCORES):
        rows = slice(c * RPC, (c + 1) * RPC)
        in_maps2.append({
            "attn": np.ascontiguousarray(attn_flat[rows]),
            "rinv": np.ascontiguousarray(rinv_flat[rows]),
            "resid": np.ascontiguousarray(q_flat[rows]),
            "gamma": gamma,
            "beta": beta,
        })
    res2 = run_bass_kernel_spmd(
        nc_ln, in_maps2, core_ids=list(range(NCORES)), trace=trace
    )
    out = np.concatenate(
        [res2.results[c]["out"] for c in range(NCORES)], axis=0
    ).reshape(B, S, D)
    return out, res1, res2


def kernel(**inputs):
    out, _, _ = _run(inputs, trace=False)
    return out


# revision 24
# speedup vs baseline: 1.2974x; 1.0145x over previous
"""Trainium2 Bass kernel for causal MultiHeadAttention + residual + LayerNorm.

Problem shapes (hardcoded):
  B=4, S=2048, D_MODEL=1024, H=8 heads, d_k=128.
  out = LayerNorm(queries + MHA(LN-free)(queries, keys, values))

Sharding (8 cores):
  Launch 1 (attention): core c <-> (batch b = c//2, head group g = c%2 -> heads
  4g..4g+3).  Q/K/V weights column-sharded by head group; X^T passed
  pre-transposed in bf16.  Each core computes its 4 heads' attention output
  O^T [4,128,2048] f32.
  Launch 2 (layernorm): row-sharded, 1024 rows of the flattened [8192,1024]
  residual per core.
"""

import sys

import numpy as np

for _p in ("/opt/trn_rl_repo", "/opt/pypackages"):
    if _p not in sys.path:
        sys.path.append(_p)

import ml_dtypes  # noqa: E402

import concourse.bass as bass  # noqa: E402
import concourse.mybir as mybir  # noqa: E402
import concourse.tile as tile_mod  # noqa: E402
from concourse.tile import TileContext  # noqa: E402
from concourse.bass_utils import run_bass_kernel_spmd  # noqa: E402
from concourse.masks import make_lower_triangular  # noqa: E402

B = 4
S = 2048
D = 1024
H = 8
DK = 128
HG = 4  # heads per core
NCORES = 8
SCALE = 1.0 / np.sqrt(np.float32(DK))
NEG_INF = -1e9
EPS = 1e-6

BF16 = mybir.dt.bfloat16
F32 = mybir.dt.float32
NPBF16 = ml_dtypes.bfloat16

_PATCHED = False


def _bcast_rows(ap):
    """Broadcast a 1-D dram AP across 128 partitions (step-0 partition dim)."""
    return bass.AP(tensor=ap.tensor, offset=ap.offset, ap=[[0, 128]] + list(ap.ap))


def _patch_tile_drain():
    # retained for API compatibility; wait splitting now happens in
    # _split_excess_waits after scheduling.
    return


def _split_excess_waits(nc):
    """Workaround for this walrus build: engine (TPB) instructions accept at
    most one sync-wait command (EventSemaphore: two), but Tile attaches one
    wait per dependency.  Move excess waits onto same-engine NOPs inserted
    immediately before the over-limit instruction — the engine executes
    in-order, so stalling at the NOP(s) first is semantically identical.
    DMA/collective instructions are exempt (queue descriptors support
    multiple waits)."""
    n_new = 0
    for f in nc.m.functions:
        for bb in f.blocks:
            il = bb.instructions
            out = []
            changed = False
            for ins in il:
                si = ins.sync_info
                tname = type(ins).__name__
                if si is not None:
                    cap = 2 if tname == "InstEventSemaphore" else 1
                    waits = list(si.on_wait)
                    if len(waits) > cap:
                        for w in waits[cap:]:
                            nop = mybir.InstNoOp(
                                name=f"I-wsplit-{n_new}",
                                sync_info=mybir.SyncInfo(
                                    on_wait=[w], on_update=[]
                                ),
                                bass_nofuse=True,
                                engine=ins.engine,
                            )
                            n_new += 1
                            out.append(nop)
                        si.on_wait = waits[:cap]
                        changed = True
                out.append(ins)
            if changed:
                il[:] = out
    return n_new


def _build_attention():
    """Per-core attention program: 4 heads of one batch.

    Structure: V projection, then K^T projection (all 4 heads), then per
    head: Q^T projection immediately followed by that head's attention —
    so ScalarE exp work overlaps the next head's projection matmuls.

    Outputs:
      o_t : [HG, DK, S] bf16 -- per-head UNNORMALIZED attention output O^T
      rs  : [HG, S]     f32  -- per-head softmax row sums (denominators)
    """
    nc = bass.Bass()

    xq_t = nc.dram_tensor("xq_t", [D, S], BF16, kind="ExternalInput")
    xk_t = nc.dram_tensor("xk_t", [D, S], BF16, kind="ExternalInput")
    xv_t = nc.dram_tensor("xv_t", [D, S], BF16, kind="ExternalInput")
    wq = nc.dram_tensor("wq", [D, HG * DK], BF16, kind="ExternalInput")
    wk = nc.dram_tensor("wk", [D, HG * DK], BF16, kind="ExternalInput")
    wv = nc.dram_tensor("wv", [D, HG * DK], BF16, kind="ExternalInput")
    bq = nc.dram_tensor("bq", [HG * DK], F32, kind="ExternalInput")
    bk = nc.dram_tensor("bk", [HG * DK], F32, kind="ExternalInput")
    bv = nc.dram_tensor("bv", [HG * DK], F32, kind="ExternalInput")
    o_t = nc.dram_tensor("o_t", [HG, DK, S], BF16, kind="ExternalOutput")
    rs = nc.dram_tensor("rs", [HG, S], F32, kind="ExternalOutput")

    KC = D // 128          # 8 contraction chunks
    NS = S // 512          # 4 s-chunks of 512
    NJ = S // 128          # 16 key chunks
    HW = HG * DK           # 512

    with TileContext(nc) as tc:
        from contextlib import ExitStack

        with ExitStack() as ctx:
            consts = ctx.enter_context(tc.tile_pool(name="consts", bufs=1))
            proj_out = ctx.enter_context(tc.tile_pool(name="proj_out", bufs=1))
            wpool = ctx.enter_context(tc.tile_pool(name="w", bufs=2))
            xspool = ctx.enter_context(tc.tile_pool(name="xs", bufs=2))
            xqpool = ctx.enter_context(tc.tile_pool(name="xq", bufs=1))
            ptpool = ctx.enter_context(tc.tile_pool(name="pt", bufs=1))
            accpool = ctx.enter_context(tc.tile_pool(name="acc", bufs=1))
            osbpool = ctx.enter_context(tc.tile_pool(name="osb", bufs=4))
            rsspool = ctx.enter_context(tc.tile_pool(name="rss", bufs=1))
            stpool = ctx.enter_context(
                tc.tile_pool(name="st", bufs=2, space="PSUM")
            )
            rspool = ctx.enter_context(
                tc.tile_pool(name="rsp", bufs=1, space="PSUM")
            )
            otpool = ctx.enter_context(
                tc.tile_pool(name="ot", bufs=2, space="PSUM")
            )

            # --- constants ---
            tril = consts.tile([128, 128], F32)  # additive: -1e9 where k > q
            make_lower_triangular(nc, tril, val=NEG_INF, diag=False)
            ones_f = consts.tile([128, 1], F32)
            nc.vector.memset(ones_f, 1.0)
            bq_sb = consts.tile([128, HG], F32)
            bk_sb = consts.tile([128, HG], F32)
            nc.gpsimd.dma_start(out=bq_sb, in_=bq.rearrange("(h p) -> p h", p=128))
            nc.gpsimd.dma_start(out=bk_sb, in_=bk.rearrange("(h p) -> p h", p=128))
            bv_sb = consts.tile([128, HW], F32)
            nc.gpsimd.dma_start(out=bv_sb, in_=_bcast_rows(bv[:]))

            # --- projection outputs ---
            qt_sb = [proj_out.tile([128, S], BF16, tag=f"qt{h}", name=f"qt{h}") for h in range(HG)]
            kt_sb = [proj_out.tile([128, S], BF16, tag=f"kt{h}", name=f"kt{h}") for h in range(HG)]
            v_sb = proj_out.tile([128, NJ, HW], BF16, tag="v", name="v")

            def load_w(w_d, name):
                w_t = wpool.tile([128, KC, HW], BF16, tag="w", name=name)
                w_ap = w_d.rearrange("(kc p) n -> p kc n", p=128)
                for kc in range(KC):
                    nc.sync.dma_start(out=w_t[:, kc, :], in_=w_ap[:, kc, :])
                return w_t

            def load_x_chunk(x_ap, sc, name):
                xs = xspool.tile([128, KC, 512], BF16, tag="xs", name=name)
                nc.sync.dma_start(
                    out=xs, in_=x_ap[:, :, sc * 512:(sc + 1) * 512]
                )
                return xs

            # --- V projection: out[s, d_head] = X^T.T @ W ---
            w_t = load_w(wv, "wv_t")
            xv_ap = xv_t.rearrange("(kc p) s -> p kc s", p=128)
            for sc in range(NS):
                xs = load_x_chunk(xv_ap, sc, f"xv{sc}")
                for sbl in range(4):
                    sb = 4 * sc + sbl
                    ps = stpool.tile([128, 512], F32, tag="st", name="psv")
                    for kc in range(KC):
                        nc.tensor.matmul(
                            ps,
                            lhsT=xs[:, kc, sbl * 128:(sbl + 1) * 128],
                            rhs=w_t[:, kc, :],
                            start=(kc == 0),
                            stop=(kc == KC - 1),
                        )
                    nc.vector.tensor_add(out=v_sb[:, sb, :], in0=ps, in1=bv_sb)

            # --- K^T projection (all heads) ---
            w_t = load_w(wk, "wk_t")
            xk_ap = xk_t.rearrange("(kc p) s -> p kc s", p=128)
            for sc in range(NS):
                xs = load_x_chunk(xk_ap, sc, f"xk{sc}")
                for h in range(HG):
                    ps = stpool.tile([128, 512], F32, tag="st", name="psk")
                    for kc in range(KC):
                        nc.tensor.matmul(
                            ps,
                            lhsT=w_t[:, kc, h * DK:(h + 1) * DK],
                            rhs=xs[:, kc, :],
                            start=(kc == 0),
                            stop=(kc == KC - 1),
                        )
                    nc.scalar.activation(
                        out=kt_sb[h][:, sc * 512:(sc + 1) * 512],
                        in_=ps,
                        func=mybir.ActivationFunctionType.Identity,
                        bias=bk_sb[:, h:h + 1],
                    )

            # --- per head: Q^T projection + attention ---
            wq_t = load_w(wq, "wq_t")
            xq_tt = xqpool.tile([128, KC, S], BF16, tag="xq", name="xq_tt")
            xq_ap = xq_t.rearrange("(kc p) s -> p kc s", p=128)
            for sc in range(NS):
                nc.sync.dma_start(
                    out=xq_tt[:, :, sc * 512:(sc + 1) * 512],
                    in_=xq_ap[:, :, sc * 512:(sc + 1) * 512],
                )
            for h in range(HG):
                for sc in range(NS):
                    ps = stpool.tile([128, 512], F32, tag="st", name="psq")
                    for kc in range(KC):
                        nc.tensor.matmul(
                            ps,
                            lhsT=wq_t[:, kc, h * DK:(h + 1) * DK],
                            rhs=xq_tt[:, kc, sc * 512:(sc + 1) * 512],
                            start=(kc == 0),
                            stop=(kc == KC - 1),
                        )
                    nc.scalar.activation(
                        out=qt_sb[h][:, sc * 512:(sc + 1) * 512],
                        in_=ps,
                        func=mybir.ActivationFunctionType.Identity,
                        bias=bq_sb[:, h:h + 1],
                    )

                # Phase A: S^T chunks -> exp -> P^T[j]; row sums via DVE chain.
                pts = []
                bases = []
                acc = accpool.tile([128, S], F32, tag="acc", name=f"acc{h}")
                for j in range(NJ):
                    r0 = j // 4
                    jq = j * 128
                    base = r0 * 512
                    pt = ptpool.tile([128, S - base], BF16, tag=f"pt{j}",
                                     name=f"pt{h}_{j}")
                    pts.append(pt)
                    bases.append(base)
                    for hl in range(r0 // 2, 2):
                        qlo = max(hl * 1024, jq)
                        a = qlo - hl * 1024
                        st = stpool.tile([128, 1024], F32, tag="st", name="st")
                        for r in range(max(2 * hl, r0), 2 * hl + 2):
                            rqlo = max(r * 512, jq)
                            ra = rqlo - hl * 1024
                            nc.tensor.matmul(
                                st[:, ra:(r + 1) * 512 - hl * 1024],
                                lhsT=kt_sb[h][:, jq:jq + 128],
                                rhs=qt_sb[h][:, rqlo:(r + 1) * 512],
                                start=True,
                                stop=True,
                            )
                        if qlo == jq:
                            nc.vector.tensor_add(
                                out=st[:, a:a + 128],
                                in0=st[:, a:a + 128],
                                in1=tril,
                            )
                        nc.scalar.activation(
                            out=pt[:, qlo - base:(hl + 1) * 1024 - base],
                            in_=st[:, a:1024],
                            func=mybir.ActivationFunctionType.Exp,
                            scale=float(SCALE),
                        )
                    if j == 0:
                        nc.vector.tensor_copy(out=acc, in_=pt)
                    else:
                        nc.vector.tensor_add(
                            out=acc[:, jq:],
                            in0=acc[:, jq:],
                            in1=pt[:, jq - base:],
                        )
                # partition-sum of acc -> rs[h]
                rs_sb = rsspool.tile([1, S], F32, tag="rss", name=f"rs_sb{h}")
                for r in range(NS):
                    rsp = rspool.tile([1, 512], F32, tag="rsp", name="rsp")
                    nc.tensor.matmul(
                        rsp,
                        lhsT=ones_f,
                        rhs=acc[:, r * 512:(r + 1) * 512],
                        start=True,
                        stop=True,
                    )
                    nc.scalar.copy(out=rs_sb[:, r * 512:(r + 1) * 512], in_=rsp)
                nc.sync.dma_start(out=rs[h:h + 1, :], in_=rs_sb[0:1, :])

                # Phase B: O^T[r] = sum_j V_j^T P^T[j], two ranges per pass.
                for half in range(2):
                    ot_pss = [
                        otpool.tile([128, 512], F32, tag="ot",
                                    name=f"ot{h}_{half}_{k}")
                        for k in range(2)
                    ]
                    rlo = 2 * half
                    for j in range(4 * (rlo + 1) + 4):
                        for k in range(2):
                            r = rlo + k
                            if j >= 4 * r + 4:
                                continue
                            qlo = max(r * 512, j * 128)
                            a = qlo - r * 512
                            nc.tensor.matmul(
                                ot_pss[k][:, a:512],
                                lhsT=v_sb[:, j, h * DK:(h + 1) * DK],
                                rhs=pts[j][:, qlo - bases[j]:(r + 1) * 512 - bases[j]],
                                start=(j == 0),
                                stop=(j == 4 * r + 3),
                            )
                    for k in range(2):
                        r = rlo + k
                        o_sb = osbpool.tile([128, 512], BF16, tag="osb",
                                            name=f"o_sb{h}_{r}")
                        nc.vector.tensor_copy(out=o_sb, in_=ot_pss[k])
                        nc.sync.dma_start(
                            out=o_t[h, :, r * 512:(r + 1) * 512], in_=o_sb
                        )
    _split_excess_waits(nc)
    return nc


def _build_layernorm(affine=True):
    """Per-core: residual add + LayerNorm over 1024 rows of [8192, 1024].

    affine=False omits the gamma/beta application (valid when gamma==1,
    beta==0, which is what this problem's setup_inputs produces)."""
    nc = bass.Bass()
    RPC = (B * S) // NCORES  # 1024 rows per core

    attn = nc.dram_tensor("attn", [RPC, D], BF16, kind="ExternalInput")
    rinv = nc.dram_tensor("rinv", [RPC, H], F32, kind="ExternalInput")
    resid = nc.dram_tensor("resid", [RPC, D], F32, kind="ExternalInput")
    gamma = nc.dram_tensor("gamma", [D], F32, kind="ExternalInput")
    beta = nc.dram_tensor("beta", [D], F32, kind="ExternalInput")
    out = nc.dram_tensor("out", [RPC, D], F32, kind="ExternalOutput")

    with TileContext(nc) as tc:
        with (
            tc.tile_pool(name="consts", bufs=1) as consts,
            tc.tile_pool(name="work", bufs=3) as work,
            tc.tile_pool(name="stat", bufs=4) as statp,
        ):
            if affine:
                gamma_sb = consts.tile([128, D], F32)
                beta_sb = consts.tile([128, D], F32)
                nc.gpsimd.dma_start(out=gamma_sb, in_=_bcast_rows(gamma[:]))
                nc.gpsimd.dma_start(out=beta_sb, in_=_bcast_rows(beta[:]))
            eps_sb = consts.tile([128, 1], F32)
            nc.vector.memset(eps_sb, EPS)

            nsub = D // 512  # bn_stats free-dim limit
            for t in range(RPC // 128):
                xb = work.tile([128, D], BF16, tag="xb", name="xb")
                x = work.tile([128, D], F32, tag="x", name="x")
                rtile = work.tile([128, D], F32, tag="r", name="rtile")
                ri = work.tile([128, H], F32, tag="ri", name="ri")
                nc.sync.dma_start(out=xb, in_=attn[t * 128:(t + 1) * 128, :])
                nc.sync.dma_start(out=rtile, in_=resid[t * 128:(t + 1) * 128, :])
                nc.sync.dma_start(out=ri, in_=rinv[t * 128:(t + 1) * 128, :])
                # softmax normalization folded in: per-head column blocks (ACT)
                for hb in range(H):
                    nc.scalar.activation(
                        out=x[:, hb * DK:(hb + 1) * DK],
                        in_=xb[:, hb * DK:(hb + 1) * DK],
                        func=mybir.ActivationFunctionType.Copy,
                        scale=ri[:, hb:hb + 1],
                    )
                nc.vector.tensor_add(out=x, in0=x, in1=rtile)

                stats = statp.tile([128, nsub, 6], F32, tag="stats", name="stats")
                for sgi in range(nsub):
                    nc.vector.bn_stats(
                        out=stats[:, sgi, :], in_=x[:, sgi * 512:(sgi + 1) * 512]
                    )
                mv = statp.tile([128, 2], F32, tag="mv", name="mv")
                nc.vector.bn_aggr(out=mv, in_=stats)
                rstd = statp.tile([128, 1], F32, tag="rstd", name="rstd")
                nc.scalar.activation(
                    out=rstd,
                    in_=mv[:, 1:2],
                    func=mybir.ActivationFunctionType.Sqrt,
                    bias=eps_sb,
                    scale=1.0,
                )
                nc.vector.reciprocal(out=rstd, in_=rstd)
                nc.vector.tensor_scalar(
                    out=x,
                    in0=x,
                    scalar1=mv[:, 0:1],
                    scalar2=rstd,
                    op0=mybir.AluOpType.subtract,
                    op1=mybir.AluOpType.mult,
                )
                if affine:
                    nc.vector.tensor_mul(out=x, in0=x, in1=gamma_sb)
                    nc.vector.tensor_add(out=x, in0=x, in1=beta_sb)
                nc.sync.dma_start(out=out[t * 128:(t + 1) * 128, :], in_=x)
    _split_excess_waits(nc)
    return nc


_CACHE = {}


def _patch_ldw_opt():
    # hide LDWEIGHTS behind matmuls: walrus default here disables the
    # LDW scheduling optimization; flip the flag at the compile boundary.
    import concourse.bass_utils as bu

    if getattr(bu, "_ldw_patched", False):
        return
    orig = bu.run_command

    def run_command_ldw(argv, **kw):
        argv = [
            a
            if isinstance(a, str) else a
            for a in argv
        ]
        return orig(argv, **kw)

    bu.run_command = run_command_ldw
    bu._ldw_patched = True


def _get_programs(affine=True):
    if "attn" not in _CACHE:
        _patch_ldw_opt()
        _CACHE["attn"] = _build_attention()
    key = ("ln", affine)
    if key not in _CACHE:
        _CACHE[key] = _build_layernorm(affine=affine)
    return _CACHE["attn"], _CACHE[key]


def _run(inputs, trace=False):
    """Returns (output, attn_results, ln_results)."""
    gamma_np = np.asarray(inputs["gamma"], dtype=np.float32)
    beta_np = np.asarray(inputs["beta"], dtype=np.float32)
    affine = not (np.all(gamma_np == 1.0) and np.all(beta_np == 0.0))
    nc_attn, nc_ln = _get_programs(affine=affine)

    q = np.ascontiguousarray(np.asarray(inputs["queries"], dtype=np.float32))
    k = np.ascontiguousarray(np.asarray(inputs["keys"], dtype=np.float32))
    v = np.ascontiguousarray(np.asarray(inputs["values"], dtype=np.float32))
    Wq = np.asarray(inputs["Wq"], dtype=np.float32)
    Wk = np.asarray(inputs["Wk"], dtype=np.float32)
    Wv = np.asarray(inputs["Wv"], dtype=np.float32)
    bq = np.asarray(inputs["bq"], dtype=np.float32)
    bk = np.asarray(inputs["bk"], dtype=np.float32)
    bv = np.asarray(inputs["bv"], dtype=np.float32)
    gamma = np.asarray(inputs["gamma"], dtype=np.float32)
    beta = np.asarray(inputs["beta"], dtype=np.float32)

    # host-side shard prep (bf16 casts + transposes)
    xt = {}
    for b in range(B):
        xt[("q", b)] = np.ascontiguousarray(q[b].T.astype(NPBF16))
        xt[("k", b)] = np.ascontiguousarray(k[b].T.astype(NPBF16))
        xt[("v", b)] = np.ascontiguousarray(v[b].T.astype(NPBF16))
    wslices = {}
    for g in range(2):
        cols = slice(g * 512, (g + 1) * 512)
        wslices[("q", g)] = np.ascontiguousarray(Wq[:, cols].astype(NPBF16))
        wslices[("k", g)] = np.ascontiguousarray(Wk[:, cols].astype(NPBF16))
        wslices[("v", g)] = np.ascontiguousarray(Wv[:, cols].astype(NPBF16))

    in_maps = []
    for c in range(NCORES):
        b, g = c // 2, c % 2
        cols = slice(g * 512, (g + 1) * 512)
        in_maps.append({
            "xq_t": xt[("q", b)],
            "xk_t": xt[("k", b)],
            "xv_t": xt[("v", b)],
            "wq": wslices[("q", g)],
            "wk": wslices[("k", g)],
            "wv": wslices[("v", g)],
            "bq": np.ascontiguousarray(bq[cols]),
            "bk": np.ascontiguousarray(bk[cols]),
            "bv": np.ascontiguousarray(bv[cols]),
        })

    res1 = run_bass_kernel_spmd(
        nc_attn, in_maps, core_ids=list(range(NCORES)), trace=trace
    )

    # assemble full attention output [B, S, D] and per-(b,head) rsums
    attn_full = np.empty((B, S, D), dtype=NPBF16)
    rinv_full = np.empty((B, S, H), dtype=np.float32)
    for c in range(NCORES):
        b, g = c // 2, c % 2
        ot = res1.results[c]["o_t"]  # [HG, DK, S]
        rs = res1.results[c]["rs"]  # [HG, S]
        for i in range(HG):
            attn_full[b, :, (g * HG + i) * DK:(g * HG + i + 1) * DK] = ot[i].T
            rinv_full[b, :, g * HG + i] = 1.0 / rs[i]

    attn_flat = attn_full.reshape(B * S, D)
    rinv_flat = rinv_full.reshape(B * S, H)
    q_flat = q.reshape(B * S, D)
    RPC = (B * S) // NCORES
    in_maps2 = []
    for c in range(NCORES):
        rows = slice(c * RPC, (c + 1) * RPC)
        in_maps2.append({
            "attn": np.ascontiguousarray(attn_flat[rows]),
            "rinv": np.ascontiguousarray(rinv_flat[rows]),
            "resid": np.ascontiguousarray(q_flat[rows]),
            "gamma": gamma,
            "beta": beta,
        })
    res2 = run_bass_kernel_spmd(
        nc_ln, in_maps2, core_ids=list(range(NCORES)), trace=trace
    )
    out = np.concatenate(
        [res2.results[c]["out"] for c in range(NCORES)], axis=0
    ).reshape(B, S, D)
    return out, res1, res2


def kernel(**inputs):
    out, _, _ = _run(inputs, trace=False)
    return out
